# revision 16
# baseline (speedup 1.0000x reference)
"""Trainium2 Bass kernel for nn_DriftScene_88270167868070.

Contract: kernel(**inputs) takes FULL unsharded inputs (as produced by
setup_inputs()) and returns the FULL output (a scalar np.float32).

Strategy (8 NeuronCores, one SPMD launch):
  - Data-parallel transformer generator over the batch (64 scenes/core).
  - Big GEMMs (in-proj, QKV, V, Wo, FF1, FF2, out-proj) run as f16
    3-pass hi/lo splits: C = Ah.Wh + Al.Wh + Ah.Wl.  Each pass streams at
    1 cyc/row (vs fp32's 4), giving ~fp32-equivalent precision (~22 bits)
    at 3/4 the PE cycles; validated vs the jax fp32 reference at ~1e-3.
  - LN gamma/beta are folded into the weights/biases on the host, so the
    kernel normalizes to h0=(x-m)*rstd only.  LN variance uses an f16
    round-to-nearest x^2 (unbiased); the mean-sum stays fp32.
  - Attention (scores, softmax, attn.v) stays fp32; attention outputs are
    split to f16 and transposed on the PE in f16 (1 cyc/row).
  - Matching stage row-sharded, bf16, fully SBUF-resident (AllGather of
    bf16 xf + norms; column-softmax via AllReduce).
  - loss = mean((xf - fl32(xf + V))^2) with explicit fp32 rounding.
"""

import numpy as np
from contextlib import ExitStack

import concourse.bass as bass
import concourse.tile as tile
from concourse import bacc, mybir
from concourse.bass_utils import run_bass_kernel_spmd
from concourse.masks import make_identity
import ml_dtypes

F32 = mybir.dt.float32
F16 = mybir.dt.float16
BF16 = mybir.dt.bfloat16
AF = mybir.ActivationFunctionType
ALU = mybir.AluOpType
AX = mybir.AxisListType

# Problem dims (hardcoded per contract)
B, L, CH = 512, 32, 128
D, HEADS, DEPTH, FF = 512, 8, 4, 2048
DH = D // HEADS
LN_EPS = 1e-5
NC_ = 8                 # cores
SC = B // NC_           # 64 scenes per core
T = SC * L              # 2048 tokens per core
TB = 512                # tokens per t-block
NB = T // TB            # 4 t-blocks
NS = TB // 128          # 4 subtiles per block
KD = D // 128           # 4 d-tiles
KF = FF // 128          # 16 ff-tiles
FDIM = L * CH           # 4096 flattened feature dim
KFl = FDIM // 128       # 32 f-tiles
M_SHIFT = -20.0         # global shift for column softmax stabilization

# packed AllGather layout (bf16 element offsets): xf_nat + xn bits
AG_XFN = 0                      # xf_nat [64, 4096]
AG_XN = FDIM * SC               # xn bits: f32 [64,1] viewed as bf16 [64,2]
AG_SZ = FDIM * SC + 2 * SC      # 262272

# packed bias-column layout: [128, NCOLS] host-prepped
# idx 0..3: in_b chunks; per layer li base 4+32*li:
#   +0..7 bqkv_eff[0:1024] (Q,K), +8..11 bo, +12..27 b1_eff, +28..31 b2
# idx 4+32*DEPTH: out_b
NCOLS = 4 + 32 * DEPTH + 1


def _build_nc():
    nc = bacc.Bacc("TRN2", target_bir_lowering=False, debug=False, num_devices=NC_)

    # ---------------- I/O ----------------
    def inp(name, shape, dt=F32):
        return nc.dram_tensor(name, shape, dt, kind="ExternalInput").ap()

    epsT_h = inp("epsT_h", [128, T], F16)     # eps shard hi, [ch, tok]
    epsT_l = inp("epsT_l", [128, T], F16)
    inwT_h = inp("inwT_h", [128, D], F16)     # in_w.T hi/lo
    inwT_l = inp("inwT_l", [128, D], F16)
    wqkvT_h = inp("wqkvT_h", [DEPTH, D, 3 * D], F16)  # (Wqkv*g).T hi/lo
    wqkvT_l = inp("wqkvT_l", [DEPTH, D, 3 * D], F16)
    bqkv = inp("bqkv", [DEPTH, 3 * D])        # effective qkv bias
    woT_h = inp("woT_h", [DEPTH, D, D], F16)
    woT_l = inp("woT_l", [DEPTH, D, D], F16)
    w1T_h = inp("w1T_h", [DEPTH, D, FF], F16)  # (W1*g2).T hi/lo
    w1T_l = inp("w1T_l", [DEPTH, D, FF], F16)
    w2T_h = inp("w2T_h", [DEPTH, FF, D], F16)
    w2T_l = inp("w2T_l", [DEPTH, FF, D], F16)
    outwT_h = inp("outwT_h", [D, CH], F16)
    outwT_l = inp("outwT_l", [D, CH], F16)
    colsP = inp("colsP", [128, NCOLS])        # packed bias columns
    pT = inp("pT", [FDIM, B], BF16)           # sample_p transposed [f, scene]
    pnat = inp("pnat", [B, FDIM], BF16)       # sample_p natural
    pn_bc = inp("pn_bc", [SC, B])             # ||p_j||^2 broadcast rows
    attn_mask = inp("attn_mask", [128, 512])  # 4-scene block-diag 0/1, x4 heads
    negdiag = inp("negdiag", [SC, B])         # 1e6 at (i, SC*core + i)

    loss_part = nc.dram_tensor("loss_part", [1, 1], F32, kind="ExternalOutput").ap()

    # ---------------- DRAM scratch ----------------
    ag_in_u = nc.dram_tensor("ag_in", [AG_SZ], mybir.dt.uint16).ap()
    ag_out_u = nc.dram_tensor("ag_out", [NC_ * AG_SZ], mybir.dt.uint16,
                              addr_space="Shared").ap()
    ag_in = ag_in_u.bitcast(BF16)
    ag_out = ag_out_u.bitcast(BF16)
    ar_in = nc.dram_tensor("ar_in", [1, 2 * B], F32).ap()
    ar_out = nc.dram_tensor("ar_out", [1, 2 * B], F32, addr_space="Shared").ap()

    with tile.TileContext(nc) as tc, ExitStack() as ctx:
        # ---------------- long-lived pools (bufs is PER TAG) ----------------
        const = ctx.enter_context(tc.tile_pool(name="const", bufs=1))
        xTp = ctx.enter_context(tc.tile_pool(name="xT", bufs=1))
        rowp = ctx.enter_context(tc.tile_pool(name="rows", bufs=3))
        bcp = ctx.enter_context(tc.tile_pool(name="bc", bufs=2))
        colp = ctx.enter_context(tc.tile_pool(name="colp", bufs=1))
        col2p = ctx.enter_context(tc.tile_pool(name="col2p", bufs=4))

        ps_mm = ctx.enter_context(tc.tile_pool(name="ps_mm", bufs=2, space="PSUM"))
        ps_acc = ctx.enter_context(tc.tile_pool(name="ps_acc", bufs=4, space="PSUM"))
        ps_at = ctx.enter_context(tc.tile_pool(name="ps_at", bufs=2, space="PSUM"))

        # ---------------- constants ----------------
        ident = const.tile([128, 128], F32)
        make_identity(nc, ident[:])
        ident_bf = const.tile([128, 128], BF16)
        nc.vector.tensor_copy(ident_bf[:], ident[:])
        ident16 = const.tile([128, 128], F16)
        nc.vector.tensor_copy(ident16[:], ident[:])
        ones_col = const.tile([128, 1], F32)
        nc.vector.memset(ones_col[:], 1.0)
        ones16 = const.tile([128, 1], F16)
        nc.vector.memset(ones16[:], 1.0)
        mask4_t = const.tile([128, 512], F32)
        nc.sync.dma_start(mask4_t[:], attn_mask)
        pn_t = const.tile([SC, B], F32)
        nc.sync.dma_start(pn_t[:], pn_bc)
        nd_t = const.tile([SC, B], F32)
        nc.sync.dma_start(nd_t[:], negdiag)
        eps_col = const.tile([1, 1], F32)
        nc.vector.memset(eps_col[:], LN_EPS)
        m20_col = const.tile([SC, 1], F32)
        nc.vector.memset(m20_col[:], -M_SHIFT)
        cols = const.tile([128, NCOLS], F32)
        nc.sync.dma_start(cols[:], colsP)

        def ccol(idx):
            return cols[:, idx:idx + 1]

        # residual stream X_T: KD tiles [128, T] fp32, persistent
        xT = [xTp.tile([128, T], F32, tag=f"xT{k}", name=f"xT{k}") for k in range(KD)]

        # ============ generator scope (pools released before matching) ======
        with ExitStack() as gctx:
            hp = gctx.enter_context(tc.tile_pool(name="h", bufs=2))
            sqp = gctx.enter_context(tc.tile_pool(name="sq", bufs=2))
            bw_p = gctx.enter_context(tc.tile_pool(name="bigw", bufs=8))
            wo_p = gctx.enter_context(tc.tile_pool(name="wo", bufs=8))
            w2_p = gctx.enter_context(tc.tile_pool(name="w2", bufs=32))
            qk_p = gctx.enter_context(tc.tile_pool(name="qk", bufs=8))
            v65_p = gctx.enter_context(tc.tile_pool(name="v65", bufs=2))
            e_p = gctx.enter_context(tc.tile_pool(name="et", bufs=2))
            onat_p = gctx.enter_context(tc.tile_pool(name="onat", bufs=2))
            oT_p = gctx.enter_context(tc.tile_pool(name="oT", bufs=1))
            relu_p = gctx.enter_context(tc.tile_pool(name="relu", bufs=2))

            # ====== input projection: X_T = (eps @ in_w.T).T, f16 3-pass ====
            inw_h = bw_p.tile([128, FF], F16, tag="bigw", name="inw_h")
            inw_l = bw_p.tile([128, FF], F16, tag="bigw", name="inw_l")
            nc.sync.dma_start(inw_h[:, 0:D], inwT_h)
            nc.sync.dma_start(inw_l[:, 0:D], inwT_l)
            for b_ in range(NB):
                bsl = slice(b_ * TB, (b_ + 1) * TB)
                eps_h = sqp.tile([128, TB], F16, tag="sq", name="eps_h")
                eps_l = sqp.tile([128, TB], F16, tag="sq", name="eps_l")
                nc.sync.dma_start(eps_h[:], epsT_h[:, bsl])
                nc.sync.dma_start(eps_l[:], epsT_l[:, bsl])
                for dt_ in range(KD):
                    dsl = slice(dt_ * 128, (dt_ + 1) * 128)
                    ps = ps_mm.tile([128, TB], F32, tag="mm", name="ps")
                    nc.tensor.matmul(ps[:], inw_h[:, dsl], eps_h[:], start=True, stop=False)
                    nc.tensor.matmul(ps[:], inw_h[:, dsl], eps_l[:], start=False, stop=False)
                    nc.tensor.matmul(ps[:], inw_l[:, dsl], eps_h[:], start=False, stop=True)
                    nc.scalar.activation(xT[dt_][:, bsl], ps[:],
                                         AF.Identity, bias=ccol(dt_), scale=1.0)

            # ========= LN stats wave: rstd/shift broadcasts per block =========
            def ln_stats(b_, ps_pool, ps_tag):
                """PE stat sums + row chain + broadcasts for tokens of block
                b_.  Returns [128, 2*TB] tile: [:, :TB]=rstd, [:, TB:]=shift."""
                bsl = slice(b_ * TB, (b_ + 1) * TB)
                s_row = rowp.tile([1, TB], F32, tag="srow", name="srow")[:]
                q_row = rowp.tile([1, TB], F32, tag="qrow", name="qrow")[:]
                msq = rowp.tile([1, TB], F32, tag="msq", name="msq")[:]
                ps_s = ps_pool.tile([1, TB], F32, tag=ps_tag, name="ps_s")
                for k in range(KD):
                    nc.tensor.matmul(ps_s[:], ones_col[:], xT[k][:, bsl],
                                     start=(k == 0), stop=(k == KD - 1))
                nc.vector.tensor_scalar_mul(s_row, ps_s[:], -1.0 / D)   # -mean
                ps_q = ps_pool.tile([1, TB], F32, tag=ps_tag, name="ps_q")
                for k in range(KD):
                    sq = sqp.tile([128, TB], F16, tag="sq", name="sq")
                    nc.vector.tensor_mul(sq[:], xT[k][:, bsl], xT[k][:, bsl])
                    nc.tensor.matmul(ps_q[:], ones16[:], sq[:],
                                     start=(k == 0), stop=(k == KD - 1))
                nc.vector.tensor_mul(msq, s_row, s_row)
                # var = q/D - m^2  (into q_row)
                nc.vector.scalar_tensor_tensor(q_row, ps_q[:], 1.0 / D, msq,
                                               op0=ALU.mult, op1=ALU.subtract)
                # rstd = 1/sqrt(var + eps): sqrt into msq, recip into q_row
                nc.scalar.activation(msq, q_row, AF.Sqrt, bias=eps_col[:], scale=1.0)
                nc.vector.reciprocal(q_row, msq)
                # shift = -m * rstd (into s_row)
                nc.vector.tensor_mul(s_row, s_row, q_row)
                bc = bcp.tile([128, 2 * TB], F32, tag="lnbc", name="lnbc")
                nc.gpsimd.partition_broadcast(bc[:, 0:TB], q_row)
                nc.gpsimd.partition_broadcast(bc[:, TB:2 * TB], s_row)
                return bc

            def ln_h(b_, bc, htag):
                """h0 = (x - m)*rstd f16 hi/lo tile pairs for block b_."""
                bsl = slice(b_ * TB, (b_ + 1) * TB)
                hs = []
                for k in range(KD):
                    hf = hp.tile([128, TB], F32, tag="hf", name=f"{htag}f")
                    nc.vector.tensor_mul(hf[:], xT[k][:, bsl], bc[:, 0:TB])
                    nc.vector.tensor_add(hf[:], hf[:], bc[:, TB:2 * TB])
                    hh = hp.tile([128, TB], F16, tag=f"{htag}h{k}", name=f"{htag}h")
                    nc.scalar.activation(hh[:], hf[:], AF.Identity, bias=0.0, scale=1.0)
                    hl = hp.tile([128, TB], F16, tag=f"{htag}l{k}", name=f"{htag}l")
                    nc.vector.tensor_sub(hl[:], hf[:], hh[:])
                    hs.append((hh, hl))
                return hs

            # ========= transformer layers =========
            for li in range(DEPTH):
                cb = 4 + 32 * li  # packed-column base for this layer
                # ---- attention phase ----
                wq_h, wq_l = [], []
                for k in range(KD):
                    wh = bw_p.tile([128, FF], F16, tag="bigw", name="wqh")
                    nc.sync.dma_start(wh[:, 0:3 * D], wqkvT_h[li, k * 128:(k + 1) * 128, :])
                    wq_h.append(wh)
                    wl = bw_p.tile([128, FF], F16, tag="bigw", name="wql")
                    nc.sync.dma_start(wl[:, 0:3 * D], wqkvT_l[li, k * 128:(k + 1) * 128, :])
                    wq_l.append(wl)
                wo_h, wo_l = [], []
                for k in range(KD):
                    wh = wo_p.tile([128, D], F16, tag="wo", name="woh")
                    nc.sync.dma_start(wh[:], woT_h[li, k * 128:(k + 1) * 128, :])
                    wo_h.append(wh)
                    wl = wo_p.tile([128, D], F16, tag="wo", name="wol")
                    nc.sync.dma_start(wl[:], woT_l[li, k * 128:(k + 1) * 128, :])
                    wo_l.append(wl)
                bv_bc = bcp.tile([128, D], F32, tag="bvbc", name="bvbc", bufs=1)
                nc.gpsimd.dma_start(bv_bc[:], bass.AP(
                    tensor=bqkv.tensor, offset=bqkv.offset + li * 3 * D + 2 * D,
                    ap=[[0, 128], [1, D]]))

                # rolling stats: block b+1's stats issue early in block b
                bc_cur = ln_stats(0, ps_acc, "acc")
                for b_ in range(NB):
                    tsl = slice(b_ * TB, (b_ + 1) * TB)
                    h = ln_h(b_, bc_cur, "h")
                    if b_ + 1 < NB:
                        bc_cur = ln_stats(b_ + 1, ps_acc, "acc")
                    # oT tiles for this block: [2k]=hi, [2k+1]=lo per d-tile
                    oT_tiles = [oT_p.tile([128, TB], F16, tag=f"oT{j}", name="oT")
                                for j in range(2 * KD)]
                    # Q,K projections (transposed out), f16 3-pass
                    qk = []
                    for ot in range(8):
                        osl = slice(ot * 128, (ot + 1) * 128)
                        ps = ps_mm.tile([128, TB], F32, tag="mm", name="ps")
                        for k in range(KD):
                            nc.tensor.matmul(ps[:], wq_h[k][:, osl], h[k][0][:],
                                             start=(k == 0), stop=False)
                        for k in range(KD):
                            nc.tensor.matmul(ps[:], wq_h[k][:, osl], h[k][1][:],
                                             start=False, stop=False)
                        for k in range(KD):
                            nc.tensor.matmul(ps[:], wq_l[k][:, osl], h[k][0][:],
                                             start=False, stop=(k == KD - 1))
                        t = qk_p.tile([128, TB], F32, tag="qk", name="qk")
                        nc.scalar.activation(t[:], ps[:], AF.Identity,
                                             bias=ccol(cb + ot), scale=1.0)
                        qk.append(t)
                    for tt in range(NS):
                        ssl = slice(tt * 128, (tt + 1) * 128)
                        # V natural for this subtile, 65-strided with ones column
                        ps = ps_mm.tile([128, D], F32, tag="mm", name="ps")
                        for k in range(KD):
                            nc.tensor.matmul(ps[:], h[k][0][:, ssl], wq_h[k][:, 2 * D:3 * D],
                                             start=(k == 0), stop=False)
                        for k in range(KD):
                            nc.tensor.matmul(ps[:], h[k][0][:, ssl], wq_l[k][:, 2 * D:3 * D],
                                             start=False, stop=False)
                        for k in range(KD):
                            nc.tensor.matmul(ps[:], h[k][1][:, ssl], wq_h[k][:, 2 * D:3 * D],
                                             start=False, stop=(k == KD - 1))
                        v = v65_p.tile([128, 8 * 65], F32, tag="v65", name="v65")
                        nc.vector.memset(
                            v[:].rearrange("p (hh c) -> p hh c", hh=8)[:, :, 64:65], 1.0)
                        for hh in range(8):
                            nc.vector.tensor_add(v[:, hh * 65:hh * 65 + 64],
                                                 ps[:, hh * 64:(hh + 1) * 64],
                                                 bv_bc[:, hh * 64:(hh + 1) * 64])
                        # attention (fp32), per head
                        onat = onat_p.tile([128, D], F32, tag="onat", name="onat")
                        for hh in range(8):
                            bp = (hh % 2) * 64
                            kt = qk[4 + hh // 2]
                            qt = qk[hh // 2]
                            s_ps = ps_at.tile([128, 128], F32, tag="at", name="sps")
                            nc.tensor.matmul(s_ps[:], kt[bp:bp + 64, tt * 128:(tt + 1) * 128],
                                             qt[bp:bp + 64, tt * 128:(tt + 1) * 128],
                                             start=True, stop=True)
                            et = e_p.tile([128, 128], F32, tag="et", name="et")
                            nc.scalar.activation(et[:], s_ps[:], AF.Exp, bias=0.0, scale=0.125)
                            nc.vector.tensor_mul(et[:], et[:], mask4_t[:, 0:128])
                            o_ps = ps_at.tile([128, 65], F32, tag="at", name="ops")
                            nc.tensor.matmul(o_ps[:], et[:], v[:, hh * 65:(hh + 1) * 65],
                                             start=True, stop=True)
                            rcol = col2p.tile([128, 1], F32, tag="rcol", name="rcol")
                            nc.vector.reciprocal(rcol[:], o_ps[:, 64:65])
                            nc.vector.tensor_scalar_mul(onat[:, hh * 64:(hh + 1) * 64],
                                                        o_ps[:, 0:64], rcol[:])
                        # split onat to f16 hi/lo, transpose in f16
                        on_h = onat_p.tile([128, D], F16, tag="on_h", name="on_h")
                        nc.scalar.activation(on_h[:], onat[:], AF.Identity, bias=0.0, scale=1.0)
                        on_l = onat_p.tile([128, D], F16, tag="on_l", name="on_l")
                        nc.vector.tensor_sub(on_l[:], onat[:], on_h[:])
                        for k in range(KD):
                            ksl = slice(k * 128, (k + 1) * 128)
                            tp = ps_at.tile([128, 128], F16, tag="at", name="tp")
                            nc.tensor.transpose(tp[:], on_h[:, ksl], ident16[:])
                            nc.vector.tensor_copy(oT_tiles[2 * k][:, tt * 128:(tt + 1) * 128], tp[:])
                            tp2 = ps_at.tile([128, 128], F16, tag="at", name="tp2")
                            nc.tensor.transpose(tp2[:], on_l[:, ksl], ident16[:])
                            nc.vector.tensor_copy(oT_tiles[2 * k + 1][:, tt * 128:(tt + 1) * 128], tp2[:])
                    # Wo + residual, f16 3-pass
                    for ot in range(KD):
                        osl = slice(ot * 128, (ot + 1) * 128)
                        ps = ps_mm.tile([128, TB], F32, tag="mm", name="ps")
                        for k in range(KD):
                            nc.tensor.matmul(ps[:], wo_h[k][:, osl], oT_tiles[2 * k][:],
                                             start=(k == 0), stop=False)
                        for k in range(KD):
                            nc.tensor.matmul(ps[:], wo_h[k][:, osl], oT_tiles[2 * k + 1][:],
                                             start=False, stop=False)
                        for k in range(KD):
                            nc.tensor.matmul(ps[:], wo_l[k][:, osl], oT_tiles[2 * k][:],
                                             start=False, stop=(k == KD - 1))
                        nc.vector.scalar_tensor_tensor(xT[ot][:, tsl], ps[:],
                                                       ccol(cb + 8 + ot),
                                                       xT[ot][:, tsl], op0=ALU.add, op1=ALU.add)

                # ---- FF phase ----
                w1_h, w1_l = [], []
                for k in range(KD):
                    wh = bw_p.tile([128, FF], F16, tag="bigw", name="w1h")
                    nc.sync.dma_start(wh[:, 0:FF], w1T_h[li, k * 128:(k + 1) * 128, :])
                    w1_h.append(wh)
                for k in range(KD):
                    wl = bw_p.tile([128, FF], F16, tag="bigw", name="w1l")
                    nc.sync.dma_start(wl[:, 0:FF], w1T_l[li, k * 128:(k + 1) * 128, :])
                    w1_l.append(wl)
                w2_h, w2_l = [], []
                for kf in range(KF):
                    wh = w2_p.tile([128, D], F16, tag="w2", name="w2h")
                    nc.scalar.dma_start(wh[:], w2T_h[li, kf * 128:(kf + 1) * 128, :])
                    w2_h.append(wh)
                    wl = w2_p.tile([128, D], F16, tag="w2", name="w2l")
                    nc.scalar.dma_start(wl[:], w2T_l[li, kf * 128:(kf + 1) * 128, :])
                    w2_l.append(wl)
                # rolling stats (shares the mm PSUM ring)
                bc_cur = ln_stats(0, ps_mm, "mm")
                for b_ in range(NB):
                    tsl = slice(b_ * TB, (b_ + 1) * TB)
                    h2 = ln_h(b_, bc_cur, "h")
                    if b_ + 1 < NB:
                        bc_cur = ln_stats(b_ + 1, ps_mm, "mm")
                    acc = [ps_acc.tile([128, TB], F32, tag="acc", name="facc")[:]
                           for _ in range(KD)]
                    for kf in range(KF):
                        fsl = slice(kf * 128, (kf + 1) * 128)
                        ps = ps_mm.tile([128, TB], F32, tag="mm", name="ps")
                        for k in range(KD):
                            nc.tensor.matmul(ps[:], w1_h[k][:, fsl], h2[k][0][:],
                                             start=(k == 0), stop=False)
                        for k in range(KD):
                            nc.tensor.matmul(ps[:], w1_h[k][:, fsl], h2[k][1][:],
                                             start=False, stop=False)
                        for k in range(KD):
                            nc.tensor.matmul(ps[:], w1_l[k][:, fsl], h2[k][0][:],
                                             start=False, stop=(k == KD - 1))
                        rl_h = relu_p.tile([128, TB], F16, tag="rl_h", name="rl_h")
                        nc.scalar.activation(rl_h[:], ps[:], AF.Relu,
                                             bias=ccol(cb + 12 + kf), scale=1.0)
                        zz = hp.tile([128, TB], F32, tag="hf", name="zz")
                        nc.scalar.activation(zz[:], ps[:], AF.Relu,
                                             bias=ccol(cb + 12 + kf), scale=1.0)
                        rl_l = relu_p.tile([128, TB], F16, tag="rl_l", name="rl_l")
                        nc.vector.tensor_sub(rl_l[:], zz[:], rl_h[:])
                        for ot in range(KD):
                            osl = slice(ot * 128, (ot + 1) * 128)
                            nc.tensor.matmul(acc[ot], w2_h[kf][:, osl], rl_h[:],
                                             start=(kf == 0), stop=False)
                            nc.tensor.matmul(acc[ot], w2_h[kf][:, osl], rl_l[:],
                                             start=False, stop=False)
                            nc.tensor.matmul(acc[ot], w2_l[kf][:, osl], rl_h[:],
                                             start=False, stop=(kf == KF - 1))
                    for ot in range(KD):
                        nc.vector.scalar_tensor_tensor(xT[ot][:, tsl], acc[ot],
                                                       ccol(cb + 28 + ot), xT[ot][:, tsl],
                                                       op0=ALU.add, op1=ALU.add)
        # ============ generator pools released here =========================

        # ---------------- matching-stage pools (reuse generator space) ------
        outw_p = ctx.enter_context(tc.tile_pool(name="outw", bufs=1))
        outp2 = ctx.enter_context(tc.tile_pool(name="outp2", bufs=1))
        mrow = ctx.enter_context(tc.tile_pool(name="mrow", bufs=1))
        mbcp = ctx.enter_context(tc.tile_pool(name="mbc", bufs=1))
        mtch = ctx.enter_context(tc.tile_pool(name="mtch", bufs=1))
        m2p = ctx.enter_context(tc.tile_pool(name="m2p", bufs=2))
        wT_p = ctx.enter_context(tc.tile_pool(name="wTp", bufs=8))
        pnat_p = ctx.enter_context(tc.tile_pool(name="pnat", bufs=1))
        xfa_p = ctx.enter_context(tc.tile_pool(name="xfa", bufs=1))
        xfTs_p = ctx.enter_context(tc.tile_pool(name="xfTs", bufs=2))
        pts_p = ctx.enter_context(tc.tile_pool(name="pts", bufs=4))
        xsp_p = ctx.enter_context(tc.tile_pool(name="xsp", bufs=2))

        # fp32 local xf in natural layout [scene, feature]
        xfl = outp2.tile([SC, FDIM], F32, tag="xfl", name="xfl")
        # y_T [ch, tok]: fp32 (for transposes) and bf16 (matching source)
        yT_f = outp2.tile([128, T], F32, tag="yTf")
        yT_bf = outp2.tile([128, T], BF16, tag="yTbf")

        # ========= output projection (f16 3-pass from xT splits) =========
        outw_h = outw_p.tile([128, KD * CH], F16, tag="outwh", name="outwh")
        outw_l = outw_p.tile([128, KD * CH], F16, tag="outwl", name="outwl")
        for k in range(KD):
            nc.sync.dma_start(outw_h[:, k * CH:(k + 1) * CH],
                              outwT_h[k * 128:(k + 1) * 128, :])
            nc.sync.dma_start(outw_l[:, k * CH:(k + 1) * CH],
                              outwT_l[k * 128:(k + 1) * 128, :])
        outb_col = ccol(4 + 32 * DEPTH)

        for b_ in range(NB):
            bsl = slice(b_ * TB, (b_ + 1) * TB)
            xs = []
            for k in range(KD):
                xh = xsp_p.tile([128, TB], F16, tag=f"xh{k}", name="xh")
                nc.scalar.activation(xh[:], xT[k][:, bsl], AF.Identity, bias=0.0, scale=1.0)
                xl = xsp_p.tile([128, TB], F16, tag=f"xl{k}", name="xl")
                nc.vector.tensor_sub(xl[:], xT[k][:, bsl], xh[:])
                xs.append((xh, xl))
            ps = ps_mm.tile([128, TB], F32, tag="mm", name="ps")
            for k in range(KD):
                nc.tensor.matmul(ps[:], outw_h[:, k * CH:(k + 1) * CH], xs[k][0][:],
                                 start=(k == 0), stop=False)
            for k in range(KD):
                nc.tensor.matmul(ps[:], outw_h[:, k * CH:(k + 1) * CH], xs[k][1][:],
                                 start=False, stop=False)
            for k in range(KD):
                nc.tensor.matmul(ps[:], outw_l[:, k * CH:(k + 1) * CH], xs[k][0][:],
                                 start=False, stop=(k == KD - 1))
            nc.scalar.activation(yT_f[:, bsl], ps[:], AF.Identity,
                                 bias=outb_col, scale=1.0)
            nc.vector.tensor_copy(yT_bf[:, bsl], yT_f[:, bsl])

        # y natural via PE transposes of yT_f; bf16 shards into ag_in
        for tt in range(T // 128):
            tp = ps_at.tile([128, CH], F32, tag="at", name="yn_tp")
            nc.tensor.transpose(tp[:], yT_f[:, tt * 128:(tt + 1) * 128], ident[:])
            yn = m2p.tile([128, CH], F32, tag="yn", name="yn")
            nc.vector.tensor_copy(yn[:], tp[:])
            nc.sync.dma_start(
                xfl[tt * 4:(tt + 1) * 4, :].rearrange("p (l c) -> p l c", l=L),
                yn[:])
            ynbf = m2p.tile([128, CH], BF16, tag="ynbf", name="ynbf")
            nc.vector.tensor_copy(ynbf[:], yn[:])
            nc.scalar.dma_start(
                ag_in[AG_XFN + tt * 4 * FDIM: AG_XFN + (tt + 1) * 4 * FDIM]
                .rearrange("(i l c) -> i l c", l=L, c=CH),
                ynbf[:])

        # xn = ||xf_i||^2 via gram diag (bf16 inputs, fp32 accum)
        xfT_st = yT_bf[:].rearrange("c (i l) -> c l i", l=L)   # [128, 32, 64]
        gram = ps_at.tile([SC, SC], F32, tag="at", name="gram")
        for l in range(KFl):
            nc.tensor.matmul(gram[:], xfT_st[:, l, :], xfT_st[:, l, :],
                             start=(l == 0), stop=(l == KFl - 1))
        gd = m2p.tile([SC, SC], F32, tag="gd", name="gd")
        nc.vector.tensor_mul(gd[:], gram[:], ident[0:SC, 0:SC])
        xn_col = colp.tile([SC, 1], F32, tag="xncol", name="xncol")
        nc.vector.reduce_sum(xn_col[:], gd[:], axis=AX.X)
        agi_f32 = ag_in_u.bitcast(F32)
        nc.sync.dma_start(
            agi_f32[AG_XN // 2:AG_XN // 2 + SC].rearrange("(i bb) -> i bb", bb=1),
            xn_col[:])
        nc.gpsimd.collective_compute(
            "AllGather", ALU.bypass, replica_groups=[list(range(NC_))],
            ins=[ag_in_u[:]], outs=[ag_out_u[:]])

        # preload p tiles (no dependence on AG)
        pnat_t = []
        for jt in range(4):
            t = pnat_p.tile([128, FDIM], BF16, tag=f"pn{jt}", name=f"pn{jt}")
            nc.scalar.dma_start(t[:], pnat[jt * 128:(jt + 1) * 128, :])
            pnat_t.append(t)

        # S_pos (does not need AG): acc over 32 f-chunks
        spos = ps_acc.tile([SC, B], F32, tag="acc", name="spos")
        for l in range(KFl):
            mv = pts_p.tile([128, B], BF16, tag="mv", name="mv")
            nc.sync.dma_start(mv[:], pT[l * 128:(l + 1) * 128, :])
            nc.tensor.matmul(spos[:], xfT_st[:, l, :], mv[:],
                             start=(l == 0), stop=(l == KFl - 1))

        # xn_full row [1, 512] f32 + broadcast
        ago_f32 = ag_out_u.bitcast(F32)
        xn_row = mrow.tile([1, B], F32, tag="mr", name="xnrow")
        nc.sync.dma_start(
            xn_row[:],
            bass.AP(tensor=ago_f32.tensor, offset=ago_f32.offset + AG_XN // 2,
                    ap=[[1, 1], [AG_SZ // 2, NC_], [1, SC]]))
        xn_bc = mbcp.tile([SC, B], F32, tag="mbc", name="xnbc")
        nc.gpsimd.partition_broadcast(xn_bc[:], xn_row[:])

        # xf_nat_all: 4 scene-tiles [128, 4096] bf16 (8KB lines)
        xfa = []
        for st in range(4):
            t = xfa_p.tile([128, FDIM], BF16, tag=f"xfa{st}", name=f"xfa{st}")
            for half in range(2):
                c = 2 * st + half
                nc.sync.dma_start(
                    t[half * SC:(half + 1) * SC, :],
                    bass.AP(tensor=ag_out.tensor,
                            offset=ag_out.offset + c * AG_SZ + AG_XFN,
                            ap=[[FDIM, SC], [1, FDIM]]))
            xfa.append(t)

        # S_neg: rebuild xf^T_all [128 f, 512 scene] per l-chunk via PE
        # transposes (double-buffered), accumulate immediately
        sneg = ps_acc.tile([SC, B], F32, tag="acc", name="sneg")
        for l in range(KFl):
            xfT_l = xfTs_p.tile([128, B], BF16, tag="xfTs", name="xfTs")
            for st in range(4):
                tp = ps_at.tile([128, 128], BF16, tag="at", name="ttp")
                nc.tensor.transpose(tp[:], xfa[st][:, l * 128:(l + 1) * 128],
                                    ident_bf[:])
                nc.vector.tensor_copy(xfT_l[:, st * 128:(st + 1) * 128], tp[:])
            nc.tensor.matmul(sneg[:], xfT_st[:, l, :], xfT_l[:],
                             start=(l == 0), stop=(l == KFl - 1))

        # distances -> logits -> E (in place)
        dist = mtch.tile([SC, 2 * B], F32, tag="dist")
        nc.vector.scalar_tensor_tensor(dist[:, 0:B], spos[:], -2.0, pn_t[:],
                                       op0=ALU.mult, op1=ALU.add)
        nc.vector.scalar_tensor_tensor(dist[:, B:2 * B], sneg[:], -2.0, xn_bc[:],
                                       op0=ALU.mult, op1=ALU.add)
        nc.vector.tensor_scalar_add(dist[:], dist[:], xn_col[:])
        nc.vector.tensor_scalar_max(dist[:], dist[:], 0.0)
        nc.scalar.activation(dist[:], dist[:], AF.Sqrt, bias=0.0, scale=1.0)
        nc.vector.tensor_add(dist[:, B:2 * B], dist[:, B:2 * B], nd_t[:])
        dmin = colp.tile([SC, 1], F32, tag="dmin", name="dmin")
        nc.vector.tensor_reduce(out=dmin[:], in_=dist[:], axis=AX.X, op=ALU.min)
        E = dist  # in place: E = exp(-d + dmin)
        nc.scalar.activation(E[:], dist[:], AF.Exp, bias=dmin[:], scale=-1.0)
        g_col = colp.tile([SC, 1], F32, tag="gcol", name="gcol")
        nc.scalar.activation(g_col[:], dmin[:], AF.Exp, bias=m20_col[:], scale=-1.0)
        sr_col = colp.tile([SC, 1], F32, tag="srcol", name="srcol")
        nc.vector.reduce_sum(sr_col[:], E[:], axis=AX.X)
        # partial colsums of G = E * g_i via g-weighted stationary
        cs_row = mrow.tile([1, 2 * B], F32, tag="mr", name="csrow")
        for b_ in range(2):
            ps = ps_mm.tile([1, B], F32, tag="mm", name="ps")
            nc.tensor.matmul(ps[:], g_col[:], E[:, b_ * B:(b_ + 1) * B],
                             start=True, stop=True)
            nc.vector.tensor_copy(cs_row[:, b_ * B:(b_ + 1) * B], ps[:])
        nc.sync.dma_start(ar_in, cs_row[:])
        nc.gpsimd.collective_compute(
            "AllReduce", ALU.add, replica_groups=[list(range(NC_))],
            ins=[ar_in[:]], outs=[ar_out[:]])
        cs_g = mrow.tile([1, 2 * B], F32, tag="mr", name="csg")
        nc.sync.dma_start(cs_g[:], ar_out)
        cs_bc = mbcp.tile([SC, 2 * B], F32, tag="csbc", name="csbc")
        nc.gpsimd.partition_broadcast(cs_bc[:], cs_g[:])
        nc.scalar.activation(cs_bc[:], cs_bc[:], AF.Sqrt, bias=0.0, scale=1.0)
        nc.vector.reciprocal(cs_bc[:], cs_bc[:])
        # E' = E * invsqrt(Sc); row scalars BEFORE overwriting E with W
        nc.vector.tensor_mul(E[:], E[:], cs_bc[:])
        snp = colp.tile([SC, 1], F32, tag="snp", name="snp")
        nc.vector.reduce_sum(snp[:], E[:, B:2 * B], axis=AX.X)
        spp = colp.tile([SC, 1], F32, tag="spp", name="spp")
        nc.vector.reduce_sum(spp[:], E[:, 0:B], axis=AX.X)
        tcol = colp.tile([SC, 1], F32, tag="tcol", name="tcol")
        nc.vector.reciprocal(tcol[:], sr_col[:])
        nc.vector.tensor_mul(tcol[:], tcol[:], g_col[:])
        ccl = colp.tile([SC, 1], F32, tag="ccol", name="ccol")
        nc.scalar.activation(ccl[:], tcol[:], AF.Sqrt, bias=0.0, scale=1.0)
        alpha = colp.tile([SC, 1], F32, tag="alpha", name="alpha")
        nc.vector.tensor_mul(alpha[:], tcol[:], snp[:])
        beta = colp.tile([SC, 1], F32, tag="beta", name="beta")
        nc.vector.tensor_mul(beta[:], alpha[:], spp[:])
        nc.vector.tensor_mul(beta[:], beta[:], ccl[:])
        nc.vector.tensor_scalar_mul(beta[:], beta[:], -1.0)
        # W = E' * alpha / -beta (in place), transpose, cast bf16
        nc.vector.tensor_scalar_mul(E[:, 0:B], E[:, 0:B], alpha[:])
        nc.vector.tensor_scalar_mul(E[:, B:2 * B], E[:, B:2 * B], beta[:])
        wT = []
        for half in range(2):
            for jt in range(4):
                tp = ps_at.tile([128, SC], F32, tag="at", name="wtp")
                nc.tensor.transpose(
                    tp[:], E[:, half * B + jt * 128: half * B + (jt + 1) * 128],
                    ident[0:SC, 0:SC])
                t = wT_p.tile([128, SC], BF16, tag="wT", name="wT")
                nc.vector.tensor_copy(t[:], tp[:])
                wT.append(t)
        # V and loss: V = Wpos @ p - Wneg @ xf_full, r = xf - fl(xf + V)
        # everything SBUF-resident
        lacc = m2p.tile([SC, 16], F32, tag="lacc", name="lacc", bufs=1)
        FBW = 256
        for fb in range(FDIM // FBW):
            fsl = slice(fb * FBW, (fb + 1) * FBW)
            vps = ps_acc.tile([SC, FBW], F32, tag="acc", name="vps")
            for jt in range(4):
                nc.tensor.matmul(vps[:], wT[jt][:], pnat_t[jt][:, fsl],
                                 start=(jt == 0), stop=False)
            for jt in range(4):
                nc.tensor.matmul(vps[:], wT[4 + jt][:], xfa[jt][:, fsl],
                                 start=False, stop=(jt == 3))
            t1 = m2p.tile([SC, FBW], F32, tag="t1", name="t1")
            nc.vector.tensor_add(t1[:], xfl[:, fsl], vps[:])
            nc.vector.tensor_sub(t1[:], xfl[:, fsl], t1[:])
            nc.vector.tensor_mul(t1[:], t1[:], t1[:])
            nc.vector.reduce_sum(lacc[:, fb:fb + 1], t1[:], axis=AX.X)
        lsum = colp.tile([SC, 1], F32, tag="lsum", name="lsum")
        nc.vector.reduce_sum(lsum[:], lacc[:], axis=AX.X)
        tot = ps_mm.tile([1, 1], F32, tag="mm", name="tot")
        nc.tensor.matmul(tot[:], ones_col[0:SC, :], lsum[:], start=True, stop=True)
        tot_sb = colp.tile([1, 1], F32, tag="tot", name="totsb")
        nc.vector.tensor_copy(tot_sb[:], tot[:])
        nc.sync.dma_start(loss_part, tot_sb[:])

    nc.compile()
    return nc


_NC_CACHE = None


def _get_nc():
    global _NC_CACHE
    if _NC_CACHE is None:
        _NC_CACHE = _build_nc()
    return _NC_CACHE


def _split16(a):
    """f16 hi/lo split (round-to-nearest): a ~= hi + lo."""
    hi = a.astype(np.float16)
    lo = (a - hi.astype(np.float32)).astype(np.float16)
    return np.ascontiguousarray(hi), np.ascontiguousarray(lo)


def _prep_inputs(inputs):
    f32 = lambda x: np.ascontiguousarray(np.asarray(x), dtype=np.float32)
    bf = lambda x: np.ascontiguousarray(np.asarray(x, dtype=ml_dtypes.bfloat16))
    sample_p = f32(inputs["sample_p"])
    eps = f32(inputs["eps"])
    p2 = sample_p.reshape(B, FDIM)
    pn = (p2.astype(np.float64) ** 2).sum(-1).astype(np.float32)

    g1 = f32(inputs["ln1_g"])   # [DEPTH, D]
    b1n = f32(inputs["ln1_b"])
    g2 = f32(inputs["ln2_g"])
    b2n = f32(inputs["ln2_b"])
    Wqkv = f32(inputs["Wqkv"])  # [DEPTH, 3D, D]
    W1 = f32(inputs["W1"])      # [DEPTH, FF, D]

    # fold LN gamma into weights, LN beta into biases
    Gqkv = Wqkv * g1[:, None, :]
    bqkv_eff = f32(inputs["bqkv"]) + np.einsum('dij,dj->di', Wqkv, b1n)
    G1 = W1 * g2[:, None, :]
    b1_eff = f32(inputs["b1"]) + np.einsum('dij,dj->di', W1, b2n)

    # packed bias columns [128, NCOLS]
    colsP = np.zeros((128, NCOLS), np.float32)
    inb = f32(inputs["in_b"])
    for k in range(KD):
        colsP[:, k] = inb[k * 128:(k + 1) * 128]
    for li in range(DEPTH):
        cb = 4 + 32 * li
        for ot in range(8):
            colsP[:, cb + ot] = bqkv_eff[li, ot * 128:(ot + 1) * 128]
        for k in range(KD):
            colsP[:, cb + 8 + k] = f32(inputs["bo"])[li, k * 128:(k + 1) * 128]
            colsP[:, cb + 28 + k] = f32(inputs["b2"])[li, k * 128:(k + 1) * 128]
        for kf in range(KF):
            colsP[:, cb + 12 + kf] = b1_eff[li, kf * 128:(kf + 1) * 128]
    colsP[:, 4 + 32 * DEPTH] = f32(inputs["out_b"])

    inwT_h, inwT_l = _split16(f32(inputs["in_w"]).T.copy())
    wqkvT_h, wqkvT_l = _split16(np.ascontiguousarray(Gqkv.transpose(0, 2, 1)))
    woT_h, woT_l = _split16(np.ascontiguousarray(f32(inputs["Wo"]).transpose(0, 2, 1)))
    w1T_h, w1T_l = _split16(np.ascontiguousarray(G1.transpose(0, 2, 1)))
    w2T_h, w2T_l = _split16(np.ascontiguousarray(f32(inputs["W2"]).transpose(0, 2, 1)))
    outwT_h, outwT_l = _split16(f32(inputs["out_w"]).T.copy())

    common = {
        "inwT_h": inwT_h, "inwT_l": inwT_l,
        "wqkvT_h": wqkvT_h, "wqkvT_l": wqkvT_l,
        "bqkv": bqkv_eff,
        "woT_h": woT_h, "woT_l": woT_l,
        "w1T_h": w1T_h, "w1T_l": w1T_l,
        "w2T_h": w2T_h, "w2T_l": w2T_l,
        "outwT_h": outwT_h, "outwT_l": outwT_l,
        "colsP": colsP,
        "pT": bf(p2.T),
        "pnat": bf(p2),
        "pn_bc": np.broadcast_to(pn[None, :], (SC, B)).copy(),
        "attn_mask": np.tile(np.kron(np.eye(4, dtype=np.float32), np.ones((32, 32), np.float32)), (1, 4)),
    }
    in_maps = []
    for c in range(NC_):
        nd = np.zeros((SC, B), np.float32)
        nd[np.arange(SC), SC * c + np.arange(SC)] = 1e6
        m = dict(common)
        eT = eps[c * SC:(c + 1) * SC].reshape(T, CH).T.copy()
        eh, el = _split16(eT)
        m["epsT_h"] = eh
        m["epsT_l"] = el
        m["negdiag"] = nd
        in_maps.append(m)
    return in_maps


def kernel(**inputs) -> np.ndarray:
    nc = _get_nc()
    in_maps = _prep_inputs(inputs)
    res = run_bass_kernel_spmd(nc, in_maps, list(range(NC_)))
    total = sum(float(r["loss_part"][0, 0]) for r in res.results)
    return np.float32(total / (B * FDIM))


# revision 17
# speedup vs baseline: 1.1960x; 1.1960x over previous
"""Trainium2 Bass kernel for nn_DriftScene_88270167868070.

Contract: kernel(**inputs) takes FULL unsharded inputs (as produced by
setup_inputs()) and returns the FULL output (a scalar np.float32).

Strategy (8 NeuronCores, one SPMD launch):
  - Data-parallel transformer generator over the batch (64 scenes/core).
  - Big GEMMs (in-proj, QKV, V, Wo, FF1, FF2, out-proj) run as f16
    3-pass hi/lo splits: C = Ah.Wh + Al.Wh + Ah.Wl.  Each pass streams at
    1 cyc/row (vs fp32's 4), giving ~fp32-equivalent precision (~22 bits)
    at 3/4 the PE cycles; validated vs the jax fp32 reference at ~1e-3.
  - LN gamma/beta are folded into the weights/biases on the host, so the
    kernel normalizes to h0=(x-m)*rstd only.  LN variance uses an f16
    round-to-nearest x^2 (unbiased); the mean-sum stays fp32.
  - Attention (scores, softmax, attn.v) stays fp32; attention outputs are
    split to f16 and transposed on the PE in f16 (1 cyc/row).
  - Matching stage row-sharded, bf16, fully SBUF-resident (AllGather of
    bf16 xf + norms; column-softmax via AllReduce).
  - loss = mean((xf - fl32(xf + V))^2) with explicit fp32 rounding.
"""

import numpy as np
from contextlib import ExitStack

import concourse.bass as bass
import concourse.tile as tile
from concourse import bacc, mybir
from concourse.bass_utils import run_bass_kernel_spmd
from concourse.masks import make_identity
import ml_dtypes

F32 = mybir.dt.float32
F16 = mybir.dt.float16
BF16 = mybir.dt.bfloat16
AF = mybir.ActivationFunctionType
ALU = mybir.AluOpType
AX = mybir.AxisListType

# Problem dims (hardcoded per contract)
B, L, CH = 512, 32, 128
D, HEADS, DEPTH, FF = 512, 8, 4, 2048
DH = D // HEADS
LN_EPS = 1e-5
NC_ = 8                 # cores
SC = B // NC_           # 64 scenes per core
T = SC * L              # 2048 tokens per core
TB = 512                # tokens per t-block
NB = T // TB            # 4 t-blocks
NS = TB // 128          # 4 subtiles per block
KD = D // 128           # 4 d-tiles
KF = FF // 128          # 16 ff-tiles
FDIM = L * CH           # 4096 flattened feature dim
KFl = FDIM // 128       # 32 f-tiles
M_SHIFT = -20.0         # global shift for column softmax stabilization

# packed AllGather layout (bf16 element offsets): xf_nat + xn bits
AG_XFN = 0                      # xf_nat [64, 4096]
AG_XN = FDIM * SC               # xn bits: f32 [64,1] viewed as bf16 [64,2]
AG_SZ = FDIM * SC + 2 * SC      # 262272

# packed bias-column layout: [128, NCOLS] host-prepped
# idx 0..3: in_b chunks; per layer li base 4+32*li:
#   +0..7 bqkv_eff[0:1024] (Q,K), +8..11 bo, +12..27 b1_eff, +28..31 b2
# idx 4+32*DEPTH: out_b
NCOLS = 4 + 32 * DEPTH + 1


def _build_nc():
    nc = bacc.Bacc("TRN2", target_bir_lowering=False, debug=False, num_devices=NC_)

    # ---------------- I/O ----------------
    def inp(name, shape, dt=F32):
        return nc.dram_tensor(name, shape, dt, kind="ExternalInput").ap()

    epsT_h = inp("epsT_h", [128, T], F16)     # eps shard hi, [ch, tok]
    epsT_l = inp("epsT_l", [128, T], F16)
    inwT_h = inp("inwT_h", [128, D], F16)     # in_w.T hi/lo
    inwT_l = inp("inwT_l", [128, D], F16)
    wqkvT_h = inp("wqkvT_h", [DEPTH, D, 3 * D], F16)  # (Wqkv*g).T hi/lo
    wqkvT_l = inp("wqkvT_l", [DEPTH, D, 3 * D], F16)
    bqkv = inp("bqkv", [DEPTH, 3 * D])        # effective qkv bias
    woT_h = inp("woT_h", [DEPTH, D, D], F16)
    woT_l = inp("woT_l", [DEPTH, D, D], F16)
    w1T_h = inp("w1T_h", [DEPTH, D, FF], F16)  # (W1*g2).T hi/lo
    w1T_l = inp("w1T_l", [DEPTH, D, FF], F16)
    w2T_h = inp("w2T_h", [DEPTH, FF, D], F16)
    w2T_l = inp("w2T_l", [DEPTH, FF, D], F16)
    outwT_h = inp("outwT_h", [D, CH], F16)
    outwT_l = inp("outwT_l", [D, CH], F16)
    colsP = inp("colsP", [128, NCOLS])        # packed bias columns
    pT = inp("pT", [FDIM, B], BF16)           # sample_p transposed [f, scene]
    pnat = inp("pnat", [B, FDIM], BF16)       # sample_p natural
    pn_bc = inp("pn_bc", [SC, B])             # ||p_j||^2 broadcast rows
    attn_mask = inp("attn_mask", [128, 512])  # 4-scene block-diag 0/1, x4 heads
    negdiag = inp("negdiag", [SC, B])         # 1e6 at (i, SC*core + i)

    loss_part = nc.dram_tensor("loss_part", [1, 1], F32, kind="ExternalOutput").ap()

    # ---------------- DRAM scratch ----------------
    ag_in_u = nc.dram_tensor("ag_in", [AG_SZ], mybir.dt.uint16).ap()
    ag_out_u = nc.dram_tensor("ag_out", [NC_ * AG_SZ], mybir.dt.uint16,
                              addr_space="Shared").ap()
    ag_in = ag_in_u.bitcast(BF16)
    ag_out = ag_out_u.bitcast(BF16)
    ar_in = nc.dram_tensor("ar_in", [1, 2 * B], F32).ap()
    ar_out = nc.dram_tensor("ar_out", [1, 2 * B], F32, addr_space="Shared").ap()

    with tile.TileContext(nc) as tc, ExitStack() as ctx:
        # ---------------- long-lived pools (bufs is PER TAG) ----------------
        const = ctx.enter_context(tc.tile_pool(name="const", bufs=1))
        xTp = ctx.enter_context(tc.tile_pool(name="xT", bufs=1))
        rowp = ctx.enter_context(tc.tile_pool(name="rows", bufs=3))
        bcp = ctx.enter_context(tc.tile_pool(name="bc", bufs=2))
        colp = ctx.enter_context(tc.tile_pool(name="colp", bufs=1))
        col2p = ctx.enter_context(tc.tile_pool(name="col2p", bufs=4))

        ps_mm = ctx.enter_context(tc.tile_pool(name="ps_mm", bufs=2, space="PSUM"))
        ps_acc = ctx.enter_context(tc.tile_pool(name="ps_acc", bufs=4, space="PSUM"))
        ps_at = ctx.enter_context(tc.tile_pool(name="ps_at", bufs=2, space="PSUM"))

        # ---------------- constants ----------------
        ident = const.tile([128, 128], F32)
        make_identity(nc, ident[:])
        ident_bf = const.tile([128, 128], BF16)
        nc.vector.tensor_copy(ident_bf[:], ident[:])
        ident16 = const.tile([128, 128], F16)
        nc.vector.tensor_copy(ident16[:], ident[:])
        ones_col = const.tile([128, 1], F32)
        nc.vector.memset(ones_col[:], 1.0)
        ones16 = const.tile([128, 1], F16)
        nc.vector.memset(ones16[:], 1.0)
        mask4_t = const.tile([128, 512], F32)
        nc.sync.dma_start(mask4_t[:], attn_mask)
        pn_t = const.tile([SC, B], F32)
        nc.sync.dma_start(pn_t[:], pn_bc)
        nd_t = const.tile([SC, B], F32)
        nc.sync.dma_start(nd_t[:], negdiag)
        eps_col = const.tile([1, 1], F32)
        nc.vector.memset(eps_col[:], LN_EPS)
        m20_col = const.tile([SC, 1], F32)
        nc.vector.memset(m20_col[:], -M_SHIFT)
        cols = const.tile([128, NCOLS], F32)
        nc.sync.dma_start(cols[:], colsP)

        def ccol(idx):
            return cols[:, idx:idx + 1]

        # residual stream X_T: KD tiles [128, T] fp32, persistent
        xT = [xTp.tile([128, T], F32, tag=f"xT{k}", name=f"xT{k}") for k in range(KD)]

        # ============ generator scope (pools released before matching) ======
        with ExitStack() as gctx:
            hp = gctx.enter_context(tc.tile_pool(name="h", bufs=2))
            sqp = gctx.enter_context(tc.tile_pool(name="sq", bufs=2))
            bw_p = gctx.enter_context(tc.tile_pool(name="bigw", bufs=8))
            wo_p = gctx.enter_context(tc.tile_pool(name="wo", bufs=8))
            w2_p = gctx.enter_context(tc.tile_pool(name="w2", bufs=32))
            qk_p = gctx.enter_context(tc.tile_pool(name="qk", bufs=8))
            v65_p = gctx.enter_context(tc.tile_pool(name="v65", bufs=2))
            e_p = gctx.enter_context(tc.tile_pool(name="et", bufs=3))
            onat_p = gctx.enter_context(tc.tile_pool(name="onat", bufs=2))
            oT_p = gctx.enter_context(tc.tile_pool(name="oT", bufs=1))
            relu_p = gctx.enter_context(tc.tile_pool(name="relu", bufs=2))

            # ====== input projection: X_T = (eps @ in_w.T).T, f16 3-pass ====
            inw_h = bw_p.tile([128, FF], F16, tag="bigw", name="inw_h")
            inw_l = bw_p.tile([128, FF], F16, tag="bigw", name="inw_l")
            nc.sync.dma_start(inw_h[:, 0:D], inwT_h)
            nc.sync.dma_start(inw_l[:, 0:D], inwT_l)
            for b_ in range(NB):
                bsl = slice(b_ * TB, (b_ + 1) * TB)
                eps_h = sqp.tile([128, TB], F16, tag="sq", name="eps_h")
                eps_l = sqp.tile([128, TB], F16, tag="sq", name="eps_l")
                nc.sync.dma_start(eps_h[:], epsT_h[:, bsl])
                nc.sync.dma_start(eps_l[:], epsT_l[:, bsl])
                for dt_ in range(KD):
                    dsl = slice(dt_ * 128, (dt_ + 1) * 128)
                    ps = ps_mm.tile([128, TB], F32, tag="mm", name="ps")
                    nc.tensor.matmul(ps[:], inw_h[:, dsl], eps_h[:], start=True, stop=False)
                    nc.tensor.matmul(ps[:], inw_h[:, dsl], eps_l[:], start=False, stop=False)
                    nc.tensor.matmul(ps[:], inw_l[:, dsl], eps_h[:], start=False, stop=True)
                    nc.scalar.activation(xT[dt_][:, bsl], ps[:],
                                         AF.Identity, bias=ccol(dt_), scale=1.0)

            # ========= LN stats wave: rstd/shift broadcasts per block =========
            def ln_stats(b_, ps_pool, ps_tag):
                """PE stat sums + row chain + broadcasts for tokens of block
                b_.  Returns [128, 2*TB] tile: [:, :TB]=rstd, [:, TB:]=shift."""
                bsl = slice(b_ * TB, (b_ + 1) * TB)
                s_row = rowp.tile([1, TB], F32, tag="srow", name="srow")[:]
                q_row = rowp.tile([1, TB], F32, tag="qrow", name="qrow")[:]
                msq = rowp.tile([1, TB], F32, tag="msq", name="msq")[:]
                ps_s = ps_pool.tile([1, TB], F32, tag=ps_tag, name="ps_s")
                for k in range(KD):
                    nc.tensor.matmul(ps_s[:], ones_col[:], xT[k][:, bsl],
                                     start=(k == 0), stop=(k == KD - 1))
                nc.vector.tensor_scalar_mul(s_row, ps_s[:], -1.0 / D)   # -mean
                ps_q = ps_pool.tile([1, TB], F32, tag=ps_tag, name="ps_q")
                for k in range(KD):
                    sq = sqp.tile([128, TB], F16, tag="sq", name="sq")
                    nc.vector.tensor_mul(sq[:], xT[k][:, bsl], xT[k][:, bsl])
                    nc.tensor.matmul(ps_q[:], ones16[:], sq[:],
                                     start=(k == 0), stop=(k == KD - 1))
                nc.vector.tensor_mul(msq, s_row, s_row)
                # var = q/D - m^2  (into q_row)
                nc.vector.scalar_tensor_tensor(q_row, ps_q[:], 1.0 / D, msq,
                                               op0=ALU.mult, op1=ALU.subtract)
                # rstd = 1/sqrt(var + eps): sqrt into msq, recip into q_row
                nc.scalar.activation(msq, q_row, AF.Sqrt, bias=eps_col[:], scale=1.0)
                nc.vector.reciprocal(q_row, msq)
                # shift = -m * rstd (into s_row)
                nc.vector.tensor_mul(s_row, s_row, q_row)
                bc = bcp.tile([128, 2 * TB], F32, tag="lnbc", name="lnbc")
                nc.gpsimd.partition_broadcast(bc[:, 0:TB], q_row)
                nc.gpsimd.partition_broadcast(bc[:, TB:2 * TB], s_row)
                return bc

            def ln_h(b_, bc, htag):
                """h0 = (x - m)*rstd f16 hi/lo tile pairs for block b_."""
                bsl = slice(b_ * TB, (b_ + 1) * TB)
                hs = []
                for k in range(KD):
                    hf = hp.tile([128, TB], F32, tag="hf", name=f"{htag}f")
                    nc.vector.tensor_mul(hf[:], xT[k][:, bsl], bc[:, 0:TB])
                    nc.vector.tensor_add(hf[:], hf[:], bc[:, TB:2 * TB])
                    hh = hp.tile([128, TB], F16, tag=f"{htag}h{k}", name=f"{htag}h")
                    nc.scalar.activation(hh[:], hf[:], AF.Identity, bias=0.0, scale=1.0)
                    hl = hp.tile([128, TB], F16, tag=f"{htag}l{k}", name=f"{htag}l")
                    nc.vector.tensor_sub(hl[:], hf[:], hh[:])
                    hs.append((hh, hl))
                return hs

            # ========= transformer layers =========
            for li in range(DEPTH):
                cb = 4 + 32 * li  # packed-column base for this layer
                # ---- attention phase ----
                wq_h, wq_l = [], []
                for k in range(KD):
                    wh = bw_p.tile([128, FF], F16, tag="bigw", name="wqh")
                    nc.sync.dma_start(wh[:, 0:3 * D], wqkvT_h[li, k * 128:(k + 1) * 128, :])
                    wq_h.append(wh)
                    wl = bw_p.tile([128, FF], F16, tag="bigw", name="wql")
                    nc.sync.dma_start(wl[:, 0:3 * D], wqkvT_l[li, k * 128:(k + 1) * 128, :])
                    wq_l.append(wl)
                wo_h, wo_l = [], []
                for k in range(KD):
                    wh = wo_p.tile([128, D], F16, tag="wo", name="woh")
                    nc.sync.dma_start(wh[:], woT_h[li, k * 128:(k + 1) * 128, :])
                    wo_h.append(wh)
                    wl = wo_p.tile([128, D], F16, tag="wo", name="wol")
                    nc.sync.dma_start(wl[:], woT_l[li, k * 128:(k + 1) * 128, :])
                    wo_l.append(wl)
                bv_bc = bcp.tile([128, D], F32, tag="bvbc", name="bvbc", bufs=1)
                nc.gpsimd.dma_start(bv_bc[:], bass.AP(
                    tensor=bqkv.tensor, offset=bqkv.offset + li * 3 * D + 2 * D,
                    ap=[[0, 128], [1, D]]))

                # rolling stats: block b+1's stats issue early in block b
                bc_cur = ln_stats(0, ps_acc, "acc")
                for b_ in range(NB):
                    tsl = slice(b_ * TB, (b_ + 1) * TB)
                    h = ln_h(b_, bc_cur, "h")
                    if b_ + 1 < NB:
                        bc_cur = ln_stats(b_ + 1, ps_acc, "acc")
                    # oT tiles for this block: [2k]=hi, [2k+1]=lo per d-tile
                    oT_tiles = [oT_p.tile([128, TB], F16, tag=f"oT{j}", name="oT")
                                for j in range(2 * KD)]
                    # Q,K projections (transposed out), f16 3-pass
                    qk = []
                    for ot in range(8):
                        osl = slice(ot * 128, (ot + 1) * 128)
                        ps = ps_mm.tile([128, TB], F32, tag="mm", name="ps")
                        for k in range(KD):
                            nc.tensor.matmul(ps[:], wq_h[k][:, osl], h[k][0][:],
                                             start=(k == 0), stop=False)
                        for k in range(KD):
                            nc.tensor.matmul(ps[:], wq_h[k][:, osl], h[k][1][:],
                                             start=False, stop=False)
                        for k in range(KD):
                            nc.tensor.matmul(ps[:], wq_l[k][:, osl], h[k][0][:],
                                             start=False, stop=(k == KD - 1))
                        t = qk_p.tile([128, TB], F32, tag="qk", name="qk")
                        nc.scalar.activation(t[:], ps[:], AF.Identity,
                                             bias=ccol(cb + ot), scale=1.0)
                        qk.append(t)
                    for tt in range(NS):
                        ssl = slice(tt * 128, (tt + 1) * 128)
                        # V natural for this subtile, 65-strided with ones column
                        ps = ps_mm.tile([128, D], F32, tag="mm", name="ps")
                        for k in range(KD):
                            nc.tensor.matmul(ps[:], h[k][0][:, ssl], wq_h[k][:, 2 * D:3 * D],
                                             start=(k == 0), stop=False)
                        for k in range(KD):
                            nc.tensor.matmul(ps[:], h[k][0][:, ssl], wq_l[k][:, 2 * D:3 * D],
                                             start=False, stop=False)
                        for k in range(KD):
                            nc.tensor.matmul(ps[:], h[k][1][:, ssl], wq_h[k][:, 2 * D:3 * D],
                                             start=False, stop=(k == KD - 1))
                        v = v65_p.tile([128, 8 * 65], F32, tag="v65", name="v65")
                        nc.vector.memset(
                            v[:].rearrange("p (hh c) -> p hh c", hh=8)[:, :, 64:65], 1.0)
                        for hh in range(8):
                            nc.vector.tensor_add(v[:, hh * 65:hh * 65 + 64],
                                                 ps[:, hh * 64:(hh + 1) * 64],
                                                 bv_bc[:, hh * 64:(hh + 1) * 64])
                        # attention (fp32), heads software-pipelined:
                        # score(h+1) issues before attout(h) so the PE never
                        # waits on the exp/mask producer chain.
                        onat = onat_p.tile([128, D], F32, tag="onat", name="onat")
                        et_t = [None] * 8

                        def do_score(hh):
                            bp = (hh % 2) * 64
                            kt = qk[4 + hh // 2]
                            qt = qk[hh // 2]
                            s_ps = ps_at.tile([128, 128], F32, tag="at", name="sps")
                            nc.tensor.matmul(s_ps[:], kt[bp:bp + 64, tt * 128:(tt + 1) * 128],
                                             qt[bp:bp + 64, tt * 128:(tt + 1) * 128],
                                             start=True, stop=True)
                            et = e_p.tile([128, 128], F32, tag="et", name="et")
                            nc.scalar.activation(et[:], s_ps[:], AF.Exp, bias=0.0, scale=0.125)
                            nc.vector.tensor_mul(et[:], et[:], mask4_t[:, 0:128])
                            return et

                        def do_out(hh):
                            o_ps = ps_at.tile([128, 65], F32, tag="at", name="ops")
                            nc.tensor.matmul(o_ps[:], et_t[hh][:], v[:, hh * 65:(hh + 1) * 65],
                                             start=True, stop=True)
                            rcol = col2p.tile([128, 1], F32, tag="rcol", name="rcol")
                            nc.vector.reciprocal(rcol[:], o_ps[:, 64:65])
                            nc.vector.tensor_scalar_mul(onat[:, hh * 64:(hh + 1) * 64],
                                                        o_ps[:, 0:64], rcol[:])

                        for hh in range(8):
                            et_t[hh] = do_score(hh)
                            if hh >= 1:
                                do_out(hh - 1)
                        do_out(7)
                        # split onat to f16 hi/lo, transpose in f16
                        on_h = onat_p.tile([128, D], F16, tag="on_h", name="on_h")
                        nc.scalar.activation(on_h[:], onat[:], AF.Identity, bias=0.0, scale=1.0)
                        on_l = onat_p.tile([128, D], F16, tag="on_l", name="on_l")
                        nc.vector.tensor_sub(on_l[:], onat[:], on_h[:])
                        for k in range(KD):
                            ksl = slice(k * 128, (k + 1) * 128)
                            tp = ps_at.tile([128, 128], F16, tag="at", name="tp")
                            nc.tensor.transpose(tp[:], on_h[:, ksl], ident16[:])
                            nc.vector.tensor_copy(oT_tiles[2 * k][:, tt * 128:(tt + 1) * 128], tp[:])
                            tp2 = ps_at.tile([128, 128], F16, tag="at", name="tp2")
                            nc.tensor.transpose(tp2[:], on_l[:, ksl], ident16[:])
                            nc.vector.tensor_copy(oT_tiles[2 * k + 1][:, tt * 128:(tt + 1) * 128], tp2[:])
                    # Wo + residual, f16 3-pass
                    for ot in range(KD):
                        osl = slice(ot * 128, (ot + 1) * 128)
                        ps = ps_mm.tile([128, TB], F32, tag="mm", name="ps")
                        for k in range(KD):
                            nc.tensor.matmul(ps[:], wo_h[k][:, osl], oT_tiles[2 * k][:],
                                             start=(k == 0), stop=False)
                        for k in range(KD):
                            nc.tensor.matmul(ps[:], wo_h[k][:, osl], oT_tiles[2 * k + 1][:],
                                             start=False, stop=False)
                        for k in range(KD):
                            nc.tensor.matmul(ps[:], wo_l[k][:, osl], oT_tiles[2 * k][:],
                                             start=False, stop=(k == KD - 1))
                        nc.vector.scalar_tensor_tensor(xT[ot][:, tsl], ps[:],
                                                       ccol(cb + 8 + ot),
                                                       xT[ot][:, tsl], op0=ALU.add, op1=ALU.add)

                # ---- FF phase ----
                w1_h, w1_l = [], []
                for k in range(KD):
                    wh = bw_p.tile([128, FF], F16, tag="bigw", name="w1h")
                    nc.sync.dma_start(wh[:, 0:FF], w1T_h[li, k * 128:(k + 1) * 128, :])
                    w1_h.append(wh)
                for k in range(KD):
                    wl = bw_p.tile([128, FF], F16, tag="bigw", name="w1l")
                    nc.sync.dma_start(wl[:, 0:FF], w1T_l[li, k * 128:(k + 1) * 128, :])
                    w1_l.append(wl)
                w2_h, w2_l = [], []
                for kf in range(KF):
                    wh = w2_p.tile([128, D], F16, tag="w2", name="w2h")
                    nc.scalar.dma_start(wh[:], w2T_h[li, kf * 128:(kf + 1) * 128, :])
                    w2_h.append(wh)
                    wl = w2_p.tile([128, D], F16, tag="w2", name="w2l")
                    nc.scalar.dma_start(wl[:], w2T_l[li, kf * 128:(kf + 1) * 128, :])
                    w2_l.append(wl)
                # rolling stats (shares the mm PSUM ring)
                bc_cur = ln_stats(0, ps_mm, "mm")
                for b_ in range(NB):
                    tsl = slice(b_ * TB, (b_ + 1) * TB)
                    h2 = ln_h(b_, bc_cur, "h")
                    if b_ + 1 < NB:
                        bc_cur = ln_stats(b_ + 1, ps_mm, "mm")
                    acc = [ps_acc.tile([128, TB], F32, tag="acc", name="facc")[:]
                           for _ in range(KD)]
                    rl_t = [None] * KF

                    def ff1(kf):
                        fsl = slice(kf * 128, (kf + 1) * 128)
                        ps = ps_mm.tile([128, TB], F32, tag="mm", name="ps")
                        for k in range(KD):
                            nc.tensor.matmul(ps[:], w1_h[k][:, fsl], h2[k][0][:],
                                             start=(k == 0), stop=False)
                        for k in range(KD):
                            nc.tensor.matmul(ps[:], w1_h[k][:, fsl], h2[k][1][:],
                                             start=False, stop=False)
                        for k in range(KD):
                            nc.tensor.matmul(ps[:], w1_l[k][:, fsl], h2[k][0][:],
                                             start=False, stop=(k == KD - 1))
                        rl_h = relu_p.tile([128, TB], F16, tag="rl_h", name="rl_h")
                        nc.scalar.activation(rl_h[:], ps[:], AF.Relu,
                                             bias=ccol(cb + 12 + kf), scale=1.0)
                        zz = hp.tile([128, TB], F32, tag="hf", name="zz")
                        nc.vector.tensor_scalar(zz[:], ps[:], ccol(cb + 12 + kf), 0.0,
                                                op0=ALU.add, op1=ALU.max)
                        rl_l = relu_p.tile([128, TB], F16, tag="rl_l", name="rl_l")
                        nc.vector.tensor_sub(rl_l[:], zz[:], rl_h[:])
                        return (rl_h, rl_l)

                    def ff2(kf):
                        rl_h, rl_l = rl_t[kf]
                        for ot in range(KD):
                            osl = slice(ot * 128, (ot + 1) * 128)
                            nc.tensor.matmul(acc[ot], w2_h[kf][:, osl], rl_h[:],
                                             start=(kf == 0), stop=False)
                            nc.tensor.matmul(acc[ot], w2_h[kf][:, osl], rl_l[:],
                                             start=False, stop=False)
                            nc.tensor.matmul(acc[ot], w2_l[kf][:, osl], rl_h[:],
                                             start=False, stop=(kf == KF - 1))

                    for kf in range(KF):
                        rl_t[kf] = ff1(kf)
                        if kf >= 1:
                            ff2(kf - 1)
                    ff2(KF - 1)
                    for ot in range(KD):
                        nc.vector.scalar_tensor_tensor(xT[ot][:, tsl], acc[ot],
                                                       ccol(cb + 28 + ot), xT[ot][:, tsl],
                                                       op0=ALU.add, op1=ALU.add)
        # ============ generator pools released here =========================

        # ---------------- matching-stage pools (reuse generator space) ------
        outw_p = ctx.enter_context(tc.tile_pool(name="outw", bufs=1))
        outp2 = ctx.enter_context(tc.tile_pool(name="outp2", bufs=1))
        mrow = ctx.enter_context(tc.tile_pool(name="mrow", bufs=1))
        mbcp = ctx.enter_context(tc.tile_pool(name="mbc", bufs=1))
        mtch = ctx.enter_context(tc.tile_pool(name="mtch", bufs=1))
        m2p = ctx.enter_context(tc.tile_pool(name="m2p", bufs=2))
        wT_p = ctx.enter_context(tc.tile_pool(name="wTp", bufs=8))
        pnat_p = ctx.enter_context(tc.tile_pool(name="pnat", bufs=1))
        xfa_p = ctx.enter_context(tc.tile_pool(name="xfa", bufs=1))
        xfTs_p = ctx.enter_context(tc.tile_pool(name="xfTs", bufs=2))
        pts_p = ctx.enter_context(tc.tile_pool(name="pts", bufs=4))
        xsp_p = ctx.enter_context(tc.tile_pool(name="xsp", bufs=2))

        # fp32 local xf in natural layout [scene, feature]
        xfl = outp2.tile([SC, FDIM], F32, tag="xfl", name="xfl")
        # y_T [ch, tok]: fp32 (for transposes) and bf16 (matching source)
        yT_f = outp2.tile([128, T], F32, tag="yTf")
        yT_bf = outp2.tile([128, T], BF16, tag="yTbf")

        # ========= output projection (f16 3-pass from xT splits) =========
        outw_h = outw_p.tile([128, KD * CH], F16, tag="outwh", name="outwh")
        outw_l = outw_p.tile([128, KD * CH], F16, tag="outwl", name="outwl")
        for k in range(KD):
            nc.sync.dma_start(outw_h[:, k * CH:(k + 1) * CH],
                              outwT_h[k * 128:(k + 1) * 128, :])
            nc.sync.dma_start(outw_l[:, k * CH:(k + 1) * CH],
                              outwT_l[k * 128:(k + 1) * 128, :])
        outb_col = ccol(4 + 32 * DEPTH)

        for b_ in range(NB):
            bsl = slice(b_ * TB, (b_ + 1) * TB)
            xs = []
            for k in range(KD):
                xh = xsp_p.tile([128, TB], F16, tag=f"xh{k}", name="xh")
                nc.scalar.activation(xh[:], xT[k][:, bsl], AF.Identity, bias=0.0, scale=1.0)
                xl = xsp_p.tile([128, TB], F16, tag=f"xl{k}", name="xl")
                nc.vector.tensor_sub(xl[:], xT[k][:, bsl], xh[:])
                xs.append((xh, xl))
            ps = ps_mm.tile([128, TB], F32, tag="mm", name="ps")
            for k in range(KD):
                nc.tensor.matmul(ps[:], outw_h[:, k * CH:(k + 1) * CH], xs[k][0][:],
                                 start=(k == 0), stop=False)
            for k in range(KD):
                nc.tensor.matmul(ps[:], outw_h[:, k * CH:(k + 1) * CH], xs[k][1][:],
                                 start=False, stop=False)
            for k in range(KD):
                nc.tensor.matmul(ps[:], outw_l[:, k * CH:(k + 1) * CH], xs[k][0][:],
                                 start=False, stop=(k == KD - 1))
            nc.scalar.activation(yT_f[:, bsl], ps[:], AF.Identity,
                                 bias=outb_col, scale=1.0)
            nc.vector.tensor_copy(yT_bf[:, bsl], yT_f[:, bsl])

        # y natural via PE transposes of yT_f; bf16 shards into ag_in
        for tt in range(T // 128):
            tp = ps_at.tile([128, CH], F32, tag="at", name="yn_tp")
            nc.tensor.transpose(tp[:], yT_f[:, tt * 128:(tt + 1) * 128], ident[:])
            yn = m2p.tile([128, CH], F32, tag="yn", name="yn")
            nc.vector.tensor_copy(yn[:], tp[:])
            nc.sync.dma_start(
                xfl[tt * 4:(tt + 1) * 4, :].rearrange("p (l c) -> p l c", l=L),
                yn[:])
            ynbf = m2p.tile([128, CH], BF16, tag="ynbf", name="ynbf")
            nc.vector.tensor_copy(ynbf[:], yn[:])
            nc.scalar.dma_start(
                ag_in[AG_XFN + tt * 4 * FDIM: AG_XFN + (tt + 1) * 4 * FDIM]
                .rearrange("(i l c) -> i l c", l=L, c=CH),
                ynbf[:])

        # xn = ||xf_i||^2 via gram diag (bf16 inputs, fp32 accum)
        xfT_st = yT_bf[:].rearrange("c (i l) -> c l i", l=L)   # [128, 32, 64]
        gram = ps_at.tile([SC, SC], F32, tag="at", name="gram")
        for l in range(KFl):
            nc.tensor.matmul(gram[:], xfT_st[:, l, :], xfT_st[:, l, :],
                             start=(l == 0), stop=(l == KFl - 1))
        gd = m2p.tile([SC, SC], F32, tag="gd", name="gd")
        nc.vector.tensor_mul(gd[:], gram[:], ident[0:SC, 0:SC])
        xn_col = colp.tile([SC, 1], F32, tag="xncol", name="xncol")
        nc.vector.reduce_sum(xn_col[:], gd[:], axis=AX.X)
        agi_f32 = ag_in_u.bitcast(F32)
        nc.sync.dma_start(
            agi_f32[AG_XN // 2:AG_XN // 2 + SC].rearrange("(i bb) -> i bb", bb=1),
            xn_col[:])
        nc.gpsimd.collective_compute(
            "AllGather", ALU.bypass, replica_groups=[list(range(NC_))],
            ins=[ag_in_u[:]], outs=[ag_out_u[:]])

        # preload p tiles (no dependence on AG)
        pnat_t = []
        for jt in range(4):
            t = pnat_p.tile([128, FDIM], BF16, tag=f"pn{jt}", name=f"pn{jt}")
            nc.scalar.dma_start(t[:], pnat[jt * 128:(jt + 1) * 128, :])
            pnat_t.append(t)

        # S_pos (does not need AG): acc over 32 f-chunks
        spos = ps_acc.tile([SC, B], F32, tag="acc", name="spos")
        for l in range(KFl):
            mv = pts_p.tile([128, B], BF16, tag="mv", name="mv")
            nc.sync.dma_start(mv[:], pT[l * 128:(l + 1) * 128, :])
            nc.tensor.matmul(spos[:], xfT_st[:, l, :], mv[:],
                             start=(l == 0), stop=(l == KFl - 1))

        # xn_full row [1, 512] f32 + broadcast
        ago_f32 = ag_out_u.bitcast(F32)
        xn_row = mrow.tile([1, B], F32, tag="mr", name="xnrow")
        nc.sync.dma_start(
            xn_row[:],
            bass.AP(tensor=ago_f32.tensor, offset=ago_f32.offset + AG_XN // 2,
                    ap=[[1, 1], [AG_SZ // 2, NC_], [1, SC]]))
        xn_bc = mbcp.tile([SC, B], F32, tag="mbc", name="xnbc")
        nc.gpsimd.partition_broadcast(xn_bc[:], xn_row[:])

        # xf_nat_all: 4 scene-tiles [128, 4096] bf16 (8KB lines)
        xfa = []
        for st in range(4):
            t = xfa_p.tile([128, FDIM], BF16, tag=f"xfa{st}", name=f"xfa{st}")
            for half in range(2):
                c = 2 * st + half
                nc.sync.dma_start(
                    t[half * SC:(half + 1) * SC, :],
                    bass.AP(tensor=ag_out.tensor,
                            offset=ag_out.offset + c * AG_SZ + AG_XFN,
                            ap=[[FDIM, SC], [1, FDIM]]))
            xfa.append(t)

        # S_neg: rebuild xf^T_all [128 f, 512 scene] per l-chunk via PE
        # transposes (double-buffered), accumulate immediately
        sneg = ps_acc.tile([SC, B], F32, tag="acc", name="sneg")
        for l in range(KFl):
            xfT_l = xfTs_p.tile([128, B], BF16, tag="xfTs", name="xfTs")
            for st in range(4):
                tp = ps_at.tile([128, 128], BF16, tag="at", name="ttp")
                nc.tensor.transpose(tp[:], xfa[st][:, l * 128:(l + 1) * 128],
                                    ident_bf[:])
                nc.vector.tensor_copy(xfT_l[:, st * 128:(st + 1) * 128], tp[:])
            nc.tensor.matmul(sneg[:], xfT_st[:, l, :], xfT_l[:],
                             start=(l == 0), stop=(l == KFl - 1))

        # distances -> logits -> E (in place)
        dist = mtch.tile([SC, 2 * B], F32, tag="dist")
        nc.vector.scalar_tensor_tensor(dist[:, 0:B], spos[:], -2.0, pn_t[:],
                                       op0=ALU.mult, op1=ALU.add)
        nc.vector.scalar_tensor_tensor(dist[:, B:2 * B], sneg[:], -2.0, xn_bc[:],
                                       op0=ALU.mult, op1=ALU.add)
        nc.vector.tensor_scalar_add(dist[:], dist[:], xn_col[:])
        nc.vector.tensor_scalar_max(dist[:], dist[:], 0.0)
        nc.scalar.activation(dist[:], dist[:], AF.Sqrt, bias=0.0, scale=1.0)
        nc.vector.tensor_add(dist[:, B:2 * B], dist[:, B:2 * B], nd_t[:])
        dmin = colp.tile([SC, 1], F32, tag="dmin", name="dmin")
        nc.vector.tensor_reduce(out=dmin[:], in_=dist[:], axis=AX.X, op=ALU.min)
        E = dist  # in place: E = exp(-d + dmin)
        nc.scalar.activation(E[:], dist[:], AF.Exp, bias=dmin[:], scale=-1.0)
        g_col = colp.tile([SC, 1], F32, tag="gcol", name="gcol")
        nc.scalar.activation(g_col[:], dmin[:], AF.Exp, bias=m20_col[:], scale=-1.0)
        sr_col = colp.tile([SC, 1], F32, tag="srcol", name="srcol")
        nc.vector.reduce_sum(sr_col[:], E[:], axis=AX.X)
        # partial colsums of G = E * g_i via g-weighted stationary
        cs_row = mrow.tile([1, 2 * B], F32, tag="mr", name="csrow")
        for b_ in range(2):
            ps = ps_mm.tile([1, B], F32, tag="mm", name="ps")
            nc.tensor.matmul(ps[:], g_col[:], E[:, b_ * B:(b_ + 1) * B],
                             start=True, stop=True)
            nc.vector.tensor_copy(cs_row[:, b_ * B:(b_ + 1) * B], ps[:])
        nc.sync.dma_start(ar_in, cs_row[:])
        nc.gpsimd.collective_compute(
            "AllReduce", ALU.add, replica_groups=[list(range(NC_))],
            ins=[ar_in[:]], outs=[ar_out[:]])
        cs_g = mrow.tile([1, 2 * B], F32, tag="mr", name="csg")
        nc.sync.dma_start(cs_g[:], ar_out)
        cs_bc = mbcp.tile([SC, 2 * B], F32, tag="csbc", name="csbc")
        nc.gpsimd.partition_broadcast(cs_bc[:], cs_g[:])
        nc.scalar.activation(cs_bc[:], cs_bc[:], AF.Sqrt, bias=0.0, scale=1.0)
        nc.vector.reciprocal(cs_bc[:], cs_bc[:])
        # E' = E * invsqrt(Sc); row scalars BEFORE overwriting E with W
        nc.vector.tensor_mul(E[:], E[:], cs_bc[:])
        snp = colp.tile([SC, 1], F32, tag="snp", name="snp")
        nc.vector.reduce_sum(snp[:], E[:, B:2 * B], axis=AX.X)
        spp = colp.tile([SC, 1], F32, tag="spp", name="spp")
        nc.vector.reduce_sum(spp[:], E[:, 0:B], axis=AX.X)
        tcol = colp.tile([SC, 1], F32, tag="tcol", name="tcol")
        nc.vector.reciprocal(tcol[:], sr_col[:])
        nc.vector.tensor_mul(tcol[:], tcol[:], g_col[:])
        ccl = colp.tile([SC, 1], F32, tag="ccol", name="ccol")
        nc.scalar.activation(ccl[:], tcol[:], AF.Sqrt, bias=0.0, scale=1.0)
        alpha = colp.tile([SC, 1], F32, tag="alpha", name="alpha")
        nc.vector.tensor_mul(alpha[:], tcol[:], snp[:])
        beta = colp.tile([SC, 1], F32, tag="beta", name="beta")
        nc.vector.tensor_mul(beta[:], alpha[:], spp[:])
        nc.vector.tensor_mul(beta[:], beta[:], ccl[:])
        nc.vector.tensor_scalar_mul(beta[:], beta[:], -1.0)
        # W = E' * alpha / -beta (in place), transpose, cast bf16
        nc.vector.tensor_scalar_mul(E[:, 0:B], E[:, 0:B], alpha[:])
        nc.vector.tensor_scalar_mul(E[:, B:2 * B], E[:, B:2 * B], beta[:])
        wT = []
        for half in range(2):
            for jt in range(4):
                tp = ps_at.tile([128, SC], F32, tag="at", name="wtp")
                nc.tensor.transpose(
                    tp[:], E[:, half * B + jt * 128: half * B + (jt + 1) * 128],
                    ident[0:SC, 0:SC])
                t = wT_p.tile([128, SC], BF16, tag="wT", name="wT")
                nc.vector.tensor_copy(t[:], tp[:])
                wT.append(t)
        # V and loss: V = Wpos @ p - Wneg @ xf_full, r = xf - fl(xf + V)
        # everything SBUF-resident
        lacc = m2p.tile([SC, 16], F32, tag="lacc", name="lacc", bufs=1)
        FBW = 256
        for fb in range(FDIM // FBW):
            fsl = slice(fb * FBW, (fb + 1) * FBW)
            vps = ps_acc.tile([SC, FBW], F32, tag="acc", name="vps")
            for jt in range(4):
                nc.tensor.matmul(vps[:], wT[jt][:], pnat_t[jt][:, fsl],
                                 start=(jt == 0), stop=False)
            for jt in range(4):
                nc.tensor.matmul(vps[:], wT[4 + jt][:], xfa[jt][:, fsl],
                                 start=False, stop=(jt == 3))
            t1 = m2p.tile([SC, FBW], F32, tag="t1", name="t1")
            nc.vector.tensor_add(t1[:], xfl[:, fsl], vps[:])
            nc.vector.tensor_sub(t1[:], xfl[:, fsl], t1[:])
            nc.vector.tensor_mul(t1[:], t1[:], t1[:])
            nc.vector.reduce_sum(lacc[:, fb:fb + 1], t1[:], axis=AX.X)
        lsum = colp.tile([SC, 1], F32, tag="lsum", name="lsum")
        nc.vector.reduce_sum(lsum[:], lacc[:], axis=AX.X)
        tot = ps_mm.tile([1, 1], F32, tag="mm", name="tot")
        nc.tensor.matmul(tot[:], ones_col[0:SC, :], lsum[:], start=True, stop=True)
        tot_sb = colp.tile([1, 1], F32, tag="tot", name="totsb")
        nc.vector.tensor_copy(tot_sb[:], tot[:])
        nc.sync.dma_start(loss_part, tot_sb[:])

    nc.compile()
    return nc


_NC_CACHE = None


def _get_nc():
    global _NC_CACHE
    if _NC_CACHE is None:
        _NC_CACHE = _build_nc()
    return _NC_CACHE


def _split16(a):
    """f16 hi/lo split (round-to-nearest): a ~= hi + lo."""
    hi = a.astype(np.float16)
    lo = (a - hi.astype(np.float32)).astype(np.float16)
    return np.ascontiguousarray(hi), np.ascontiguousarray(lo)


def _prep_inputs(inputs):
    f32 = lambda x: np.ascontiguousarray(np.asarray(x), dtype=np.float32)
    bf = lambda x: np.ascontiguousarray(np.asarray(x, dtype=ml_dtypes.bfloat16))
    sample_p = f32(inputs["sample_p"])
    eps = f32(inputs["eps"])
    p2 = sample_p.reshape(B, FDIM)
    pn = (p2.astype(np.float64) ** 2).sum(-1).astype(np.float32)

    g1 = f32(inputs["ln1_g"])   # [DEPTH, D]
    b1n = f32(inputs["ln1_b"])
    g2 = f32(inputs["ln2_g"])
    b2n = f32(inputs["ln2_b"])
    Wqkv = f32(inputs["Wqkv"])  # [DEPTH, 3D, D]
    W1 = f32(inputs["W1"])      # [DEPTH, FF, D]

    # fold LN gamma into weights, LN beta into biases
    Gqkv = Wqkv * g1[:, None, :]
    bqkv_eff = f32(inputs["bqkv"]) + np.einsum('dij,dj->di', Wqkv, b1n)
    G1 = W1 * g2[:, None, :]
    b1_eff = f32(inputs["b1"]) + np.einsum('dij,dj->di', W1, b2n)

    # packed bias columns [128, NCOLS]
    colsP = np.zeros((128, NCOLS), np.float32)
    inb = f32(inputs["in_b"])
    for k in range(KD):
        colsP[:, k] = inb[k * 128:(k + 1) * 128]
    for li in range(DEPTH):
        cb = 4 + 32 * li
        for ot in range(8):
            colsP[:, cb + ot] = bqkv_eff[li, ot * 128:(ot + 1) * 128]
        for k in range(KD):
            colsP[:, cb + 8 + k] = f32(inputs["bo"])[li, k * 128:(k + 1) * 128]
            colsP[:, cb + 28 + k] = f32(inputs["b2"])[li, k * 128:(k + 1) * 128]
        for kf in range(KF):
            colsP[:, cb + 12 + kf] = b1_eff[li, kf * 128:(kf + 1) * 128]
    colsP[:, 4 + 32 * DEPTH] = f32(inputs["out_b"])

    inwT_h, inwT_l = _split16(f32(inputs["in_w"]).T.copy())
    wqkvT_h, wqkvT_l = _split16(np.ascontiguousarray(Gqkv.transpose(0, 2, 1)))
    woT_h, woT_l = _split16(np.ascontiguousarray(f32(inputs["Wo"]).transpose(0, 2, 1)))
    w1T_h, w1T_l = _split16(np.ascontiguousarray(G1.transpose(0, 2, 1)))
    w2T_h, w2T_l = _split16(np.ascontiguousarray(f32(inputs["W2"]).transpose(0, 2, 1)))
    outwT_h, outwT_l = _split16(f32(inputs["out_w"]).T.copy())

    common = {
        "inwT_h": inwT_h, "inwT_l": inwT_l,
        "wqkvT_h": wqkvT_h, "wqkvT_l": wqkvT_l,
        "bqkv": bqkv_eff,
        "woT_h": woT_h, "woT_l": woT_l,
        "w1T_h": w1T_h, "w1T_l": w1T_l,
        "w2T_h": w2T_h, "w2T_l": w2T_l,
        "outwT_h": outwT_h, "outwT_l": outwT_l,
        "colsP": colsP,
        "pT": bf(p2.T),
        "pnat": bf(p2),
        "pn_bc": np.broadcast_to(pn[None, :], (SC, B)).copy(),
        "attn_mask": np.tile(np.kron(np.eye(4, dtype=np.float32), np.ones((32, 32), np.float32)), (1, 4)),
    }
    in_maps = []
    for c in range(NC_):
        nd = np.zeros((SC, B), np.float32)
        nd[np.arange(SC), SC * c + np.arange(SC)] = 1e6
        m = dict(common)
        eT = eps[c * SC:(c + 1) * SC].reshape(T, CH).T.copy()
        eh, el = _split16(eT)
        m["epsT_h"] = eh
        m["epsT_l"] = el
        m["negdiag"] = nd
        in_maps.append(m)
    return in_maps


def kernel(**inputs) -> np.ndarray:
    nc = _get_nc()
    in_maps = _prep_inputs(inputs)
    res = run_bass_kernel_spmd(nc, in_maps, list(range(NC_)))
    total = sum(float(r["loss_part"][0, 0]) for r in res.results)
    return np.float32(total / (B * FDIM))


# revision 18
# speedup vs baseline: 1.2103x; 1.0120x over previous
"""Trainium2 Bass kernel for nn_DriftScene_88270167868070.

Contract: kernel(**inputs) takes FULL unsharded inputs (as produced by
setup_inputs()) and returns the FULL output (a scalar np.float32).

Strategy (8 NeuronCores, one SPMD launch):
  - Data-parallel transformer generator over the batch (64 scenes/core).
  - Big GEMMs (in-proj, QKV, V, Wo, FF1, FF2, out-proj) run as f16
    3-pass hi/lo splits: C = Ah.Wh + Al.Wh + Ah.Wl.  Each pass streams at
    1 cyc/row (vs fp32's 4), giving ~fp32-equivalent precision (~22 bits)
    at 3/4 the PE cycles; validated vs the jax fp32 reference at ~1e-3.
  - LN gamma/beta are folded into the weights/biases on the host, so the
    kernel normalizes to h0=(x-m)*rstd only.  LN variance uses an f16
    round-to-nearest x^2 (unbiased); the mean-sum stays fp32.
  - Attention (scores, softmax, attn.v) stays fp32; attention outputs are
    split to f16 and transposed on the PE in f16 (1 cyc/row).
  - Matching stage row-sharded, bf16, fully SBUF-resident (AllGather of
    bf16 xf + norms; column-softmax via AllReduce).
  - loss = mean((xf - fl32(xf + V))^2) with explicit fp32 rounding.
"""

import numpy as np
from contextlib import ExitStack

import concourse.bass as bass
import concourse.tile as tile
from concourse import bacc, mybir
from concourse.bass_utils import run_bass_kernel_spmd
from concourse.masks import make_identity
import ml_dtypes

F32 = mybir.dt.float32
F16 = mybir.dt.float16
BF16 = mybir.dt.bfloat16
AF = mybir.ActivationFunctionType
ALU = mybir.AluOpType
AX = mybir.AxisListType

# Problem dims (hardcoded per contract)
B, L, CH = 512, 32, 128
D, HEADS, DEPTH, FF = 512, 8, 4, 2048
DH = D // HEADS
LN_EPS = 1e-5
NC_ = 8                 # cores
SC = B // NC_           # 64 scenes per core
T = SC * L              # 2048 tokens per core
TB = 512                # tokens per t-block
NB = T // TB            # 4 t-blocks
NS = TB // 128          # 4 subtiles per block
KD = D // 128           # 4 d-tiles
KF = FF // 128          # 16 ff-tiles
FDIM = L * CH           # 4096 flattened feature dim
KFl = FDIM // 128       # 32 f-tiles
M_SHIFT = -20.0         # global shift for column softmax stabilization

# packed AllGather layout (bf16 element offsets): xf_nat + xn bits
AG_XFN = 0                      # xf_nat [64, 4096]
AG_XN = FDIM * SC               # xn bits: f32 [64,1] viewed as bf16 [64,2]
AG_SZ = FDIM * SC + 2 * SC      # 262272

# packed bias-column layout: [128, NCOLS] host-prepped
# idx 0..3: in_b chunks; per layer li base 4+32*li:
#   +0..7 bqkv_eff[0:1024] (Q,K), +8..11 bo, +12..27 b1_eff, +28..31 b2
# idx 4+32*DEPTH: out_b
NCOLS = 4 + 32 * DEPTH + 1


def _build_nc():
    nc = bacc.Bacc("TRN2", target_bir_lowering=False, debug=False, num_devices=NC_)

    # ---------------- I/O ----------------
    def inp(name, shape, dt=F32):
        return nc.dram_tensor(name, shape, dt, kind="ExternalInput").ap()

    epsT_h = inp("epsT_h", [128, T], F16)     # eps shard hi, [ch, tok]
    epsT_l = inp("epsT_l", [128, T], F16)
    inwT_h = inp("inwT_h", [128, D], F16)     # in_w.T hi/lo
    inwT_l = inp("inwT_l", [128, D], F16)
    wqkvT_h = inp("wqkvT_h", [DEPTH, D, 3 * D], F16)  # (Wqkv*g).T hi/lo
    wqkvT_l = inp("wqkvT_l", [DEPTH, D, 3 * D], F16)
    bqkv = inp("bqkv", [DEPTH, 3 * D])        # effective qkv bias
    woT_h = inp("woT_h", [DEPTH, D, D], F16)
    woT_l = inp("woT_l", [DEPTH, D, D], F16)
    w1T_h = inp("w1T_h", [DEPTH, D, FF], F16)  # (W1*g2).T hi/lo
    w1T_l = inp("w1T_l", [DEPTH, D, FF], F16)
    w2T_h = inp("w2T_h", [DEPTH, FF, D], F16)
    w2T_l = inp("w2T_l", [DEPTH, FF, D], F16)
    outwT_h = inp("outwT_h", [D, CH], F16)
    outwT_l = inp("outwT_l", [D, CH], F16)
    colsP = inp("colsP", [128, NCOLS])        # packed bias columns
    pT = inp("pT", [FDIM, B], BF16)           # sample_p transposed [f, scene]
    pnat = inp("pnat", [B, FDIM], BF16)       # sample_p natural
    pn_bc = inp("pn_bc", [SC, B])             # ||p_j||^2 broadcast rows
    attn_mask = inp("attn_mask", [128, 512])  # 4-scene block-diag 0/1, x4 heads
    negdiag = inp("negdiag", [SC, B])         # 1e6 at (i, SC*core + i)

    loss_part = nc.dram_tensor("loss_part", [1, 1], F32, kind="ExternalOutput").ap()

    # ---------------- DRAM scratch ----------------
    ag_in_u = nc.dram_tensor("ag_in", [AG_SZ], mybir.dt.uint16).ap()
    ag_out_u = nc.dram_tensor("ag_out", [NC_ * AG_SZ], mybir.dt.uint16,
                              addr_space="Shared").ap()
    ag_in = ag_in_u.bitcast(BF16)
    ag_out = ag_out_u.bitcast(BF16)
    ar_in = nc.dram_tensor("ar_in", [1, 2 * B], F32).ap()
    ar_out = nc.dram_tensor("ar_out", [1, 2 * B], F32, addr_space="Shared").ap()

    with tile.TileContext(nc) as tc, ExitStack() as ctx:
        # ---------------- long-lived pools (bufs is PER TAG) ----------------
        const = ctx.enter_context(tc.tile_pool(name="const", bufs=1))
        xTp = ctx.enter_context(tc.tile_pool(name="xT", bufs=1))
        rowp = ctx.enter_context(tc.tile_pool(name="rows", bufs=3))
        bcp = ctx.enter_context(tc.tile_pool(name="bc", bufs=2))
        colp = ctx.enter_context(tc.tile_pool(name="colp", bufs=1))
        col2p = ctx.enter_context(tc.tile_pool(name="col2p", bufs=4))

        ps_mm = ctx.enter_context(tc.tile_pool(name="ps_mm", bufs=2, space="PSUM"))
        ps_acc = ctx.enter_context(tc.tile_pool(name="ps_acc", bufs=4, space="PSUM"))
        ps_at = ctx.enter_context(tc.tile_pool(name="ps_at", bufs=2, space="PSUM"))

        # ---------------- constants ----------------
        ident = const.tile([128, 128], F32)
        make_identity(nc, ident[:])
        ident_bf = const.tile([128, 128], BF16)
        nc.vector.tensor_copy(ident_bf[:], ident[:])
        ident16 = const.tile([128, 128], F16)
        nc.vector.tensor_copy(ident16[:], ident[:])
        ones_col = const.tile([128, 1], F32)
        nc.vector.memset(ones_col[:], 1.0)
        ones16 = const.tile([128, 1], F16)
        nc.vector.memset(ones16[:], 1.0)
        mask4_t = const.tile([128, 512], F32)
        nc.sync.dma_start(mask4_t[:], attn_mask)
        pn_t = const.tile([SC, B], F32)
        nc.sync.dma_start(pn_t[:], pn_bc)
        nd_t = const.tile([SC, B], F32)
        nc.sync.dma_start(nd_t[:], negdiag)
        eps_col = const.tile([1, 1], F32)
        nc.vector.memset(eps_col[:], LN_EPS)
        m20_col = const.tile([SC, 1], F32)
        nc.vector.memset(m20_col[:], -M_SHIFT)
        cols = const.tile([128, NCOLS], F32)
        nc.sync.dma_start(cols[:], colsP)

        def ccol(idx):
            return cols[:, idx:idx + 1]

        # residual stream X_T: KD tiles [128, T] fp32, persistent
        xT = [xTp.tile([128, T], F32, tag=f"xT{k}", name=f"xT{k}") for k in range(KD)]

        # ============ generator scope (pools released before matching) ======
        with ExitStack() as gctx:
            hp = gctx.enter_context(tc.tile_pool(name="h", bufs=2))
            sqp = gctx.enter_context(tc.tile_pool(name="sq", bufs=2))
            bw_p = gctx.enter_context(tc.tile_pool(name="bigw", bufs=8))
            wo_p = gctx.enter_context(tc.tile_pool(name="wo", bufs=8))
            w2_p = gctx.enter_context(tc.tile_pool(name="w2", bufs=32))
            qk_p = gctx.enter_context(tc.tile_pool(name="qk", bufs=8))
            v65_p = gctx.enter_context(tc.tile_pool(name="v65", bufs=2))
            e_p = gctx.enter_context(tc.tile_pool(name="et", bufs=3))
            onat_p = gctx.enter_context(tc.tile_pool(name="onat", bufs=2))
            oT_p = gctx.enter_context(tc.tile_pool(name="oT", bufs=1))
            relu_p = gctx.enter_context(tc.tile_pool(name="relu", bufs=2))

            # ====== input projection: X_T = (eps @ in_w.T).T, f16 3-pass ====
            inw_h = bw_p.tile([128, FF], F16, tag="bigw", name="inw_h")
            inw_l = bw_p.tile([128, FF], F16, tag="bigw", name="inw_l")
            nc.sync.dma_start(inw_h[:, 0:D], inwT_h)
            nc.sync.dma_start(inw_l[:, 0:D], inwT_l)
            for b_ in range(NB):
                bsl = slice(b_ * TB, (b_ + 1) * TB)
                eps_h = sqp.tile([128, TB], F16, tag="sq", name="eps_h")
                eps_l = sqp.tile([128, TB], F16, tag="sq", name="eps_l")
                nc.sync.dma_start(eps_h[:], epsT_h[:, bsl])
                nc.sync.dma_start(eps_l[:], epsT_l[:, bsl])
                for dt_ in range(KD):
                    dsl = slice(dt_ * 128, (dt_ + 1) * 128)
                    ps = ps_mm.tile([128, TB], F32, tag="mm", name="ps")
                    nc.tensor.matmul(ps[:], inw_h[:, dsl], eps_h[:], start=True, stop=False)
                    nc.tensor.matmul(ps[:], inw_h[:, dsl], eps_l[:], start=False, stop=False)
                    nc.tensor.matmul(ps[:], inw_l[:, dsl], eps_h[:], start=False, stop=True)
                    nc.scalar.activation(xT[dt_][:, bsl], ps[:],
                                         AF.Identity, bias=ccol(dt_), scale=1.0)

            # ========= LN stats wave: rstd/shift broadcasts per block =========
            def ln_stats(b_, ps_pool, ps_tag):
                """PE stat sums + row chain + broadcasts for tokens of block
                b_.  Returns [128, 2*TB] tile: [:, :TB]=rstd, [:, TB:]=shift."""
                bsl = slice(b_ * TB, (b_ + 1) * TB)
                s_row = rowp.tile([1, TB], F32, tag="srow", name="srow")[:]
                q_row = rowp.tile([1, TB], F32, tag="qrow", name="qrow")[:]
                msq = rowp.tile([1, TB], F32, tag="msq", name="msq")[:]
                ps_s = ps_pool.tile([1, TB], F32, tag=ps_tag, name="ps_s")
                for k in range(KD):
                    nc.tensor.matmul(ps_s[:], ones_col[:], xT[k][:, bsl],
                                     start=(k == 0), stop=(k == KD - 1))
                nc.vector.tensor_scalar_mul(s_row, ps_s[:], -1.0 / D)   # -mean
                ps_q = ps_pool.tile([1, TB], F32, tag=ps_tag, name="ps_q")
                for k in range(KD):
                    sq = sqp.tile([128, TB], F16, tag="sq", name="sq")
                    nc.vector.tensor_mul(sq[:], xT[k][:, bsl], xT[k][:, bsl])
                    nc.tensor.matmul(ps_q[:], ones16[:], sq[:],
                                     start=(k == 0), stop=(k == KD - 1))
                nc.vector.tensor_mul(msq, s_row, s_row)
                # var = q/D - m^2  (into q_row)
                nc.vector.scalar_tensor_tensor(q_row, ps_q[:], 1.0 / D, msq,
                                               op0=ALU.mult, op1=ALU.subtract)
                # rstd = 1/sqrt(var + eps): sqrt into msq, recip into q_row
                nc.scalar.activation(msq, q_row, AF.Sqrt, bias=eps_col[:], scale=1.0)
                nc.vector.reciprocal(q_row, msq)
                # shift = -m * rstd (into s_row)
                nc.vector.tensor_mul(s_row, s_row, q_row)
                bc = bcp.tile([128, 2 * TB], F32, tag="lnbc", name="lnbc")
                nc.gpsimd.partition_broadcast(bc[:, 0:TB], q_row)
                nc.gpsimd.partition_broadcast(bc[:, TB:2 * TB], s_row)
                return bc

            def ln_h(b_, bc, htag):
                """h0 = (x - m)*rstd f16 hi/lo tile pairs for block b_."""
                bsl = slice(b_ * TB, (b_ + 1) * TB)
                hs = []
                for k in range(KD):
                    hf = hp.tile([128, TB], F32, tag="hf", name=f"{htag}f")
                    nc.vector.tensor_mul(hf[:], xT[k][:, bsl], bc[:, 0:TB])
                    nc.vector.tensor_add(hf[:], hf[:], bc[:, TB:2 * TB])
                    hh = hp.tile([128, TB], F16, tag=f"{htag}h{k}", name=f"{htag}h")
                    nc.scalar.activation(hh[:], hf[:], AF.Identity, bias=0.0, scale=1.0)
                    hl = hp.tile([128, TB], F16, tag=f"{htag}l{k}", name=f"{htag}l")
                    nc.vector.tensor_sub(hl[:], hf[:], hh[:])
                    hs.append((hh, hl))
                return hs

            # ========= transformer layers =========
            for li in range(DEPTH):
                cb = 4 + 32 * li  # packed-column base for this layer
                # ---- attention phase ----
                wq_h, wq_l = [], []
                for k in range(KD):
                    wh = bw_p.tile([128, FF], F16, tag="bigw", name="wqh")
                    nc.sync.dma_start(wh[:, 0:3 * D], wqkvT_h[li, k * 128:(k + 1) * 128, :])
                    wq_h.append(wh)
                    wl = bw_p.tile([128, FF], F16, tag="bigw", name="wql")
                    nc.scalar.dma_start(wl[:, 0:3 * D], wqkvT_l[li, k * 128:(k + 1) * 128, :])
                    wq_l.append(wl)
                wo_h, wo_l = [], []
                for k in range(KD):
                    wh = wo_p.tile([128, D], F16, tag="wo", name="woh")
                    nc.sync.dma_start(wh[:], woT_h[li, k * 128:(k + 1) * 128, :])
                    wo_h.append(wh)
                    wl = wo_p.tile([128, D], F16, tag="wo", name="wol")
                    nc.scalar.dma_start(wl[:], woT_l[li, k * 128:(k + 1) * 128, :])
                    wo_l.append(wl)
                bv_bc = bcp.tile([128, D], F32, tag="bvbc", name="bvbc", bufs=1)
                nc.gpsimd.dma_start(bv_bc[:], bass.AP(
                    tensor=bqkv.tensor, offset=bqkv.offset + li * 3 * D + 2 * D,
                    ap=[[0, 128], [1, D]]))

                # rolling stats: block b+1's stats issue early in block b
                bc_cur = ln_stats(0, ps_acc, "acc")
                for b_ in range(NB):
                    tsl = slice(b_ * TB, (b_ + 1) * TB)
                    h = ln_h(b_, bc_cur, "h")
                    if b_ + 1 < NB:
                        bc_cur = ln_stats(b_ + 1, ps_acc, "acc")
                    # oT tiles for this block: [2k]=hi, [2k+1]=lo per d-tile
                    oT_tiles = [oT_p.tile([128, TB], F16, tag=f"oT{j}", name="oT")
                                for j in range(2 * KD)]
                    # Q,K projections (transposed out), f16 3-pass
                    qk = []
                    for ot in range(8):
                        osl = slice(ot * 128, (ot + 1) * 128)
                        ps = ps_mm.tile([128, TB], F32, tag="mm", name="ps")
                        for k in range(KD):
                            nc.tensor.matmul(ps[:], wq_h[k][:, osl], h[k][0][:],
                                             start=(k == 0), stop=False)
                        for k in range(KD):
                            nc.tensor.matmul(ps[:], wq_h[k][:, osl], h[k][1][:],
                                             start=False, stop=False)
                        for k in range(KD):
                            nc.tensor.matmul(ps[:], wq_l[k][:, osl], h[k][0][:],
                                             start=False, stop=(k == KD - 1))
                        t = qk_p.tile([128, TB], F32, tag="qk", name="qk")
                        nc.scalar.activation(t[:], ps[:], AF.Identity,
                                             bias=ccol(cb + ot), scale=1.0)
                        qk.append(t)
                    def do_transposes(tt_, onh_, onl_):
                        for k in range(KD):
                            ksl = slice(k * 128, (k + 1) * 128)
                            tp = ps_at.tile([128, 128], F16, tag="at", name="tp")
                            nc.tensor.transpose(tp[:], onh_[:, ksl], ident16[:])
                            nc.vector.tensor_copy(
                                oT_tiles[2 * k][:, tt_ * 128:(tt_ + 1) * 128], tp[:])
                            tp2 = ps_at.tile([128, 128], F16, tag="at", name="tp2")
                            nc.tensor.transpose(tp2[:], onl_[:, ksl], ident16[:])
                            nc.vector.tensor_copy(
                                oT_tiles[2 * k + 1][:, tt_ * 128:(tt_ + 1) * 128], tp2[:])

                    pend_tp = None
                    for tt in range(NS):
                        ssl = slice(tt * 128, (tt + 1) * 128)
                        # V natural for this subtile, 65-strided with ones column
                        ps = ps_mm.tile([128, D], F32, tag="mm", name="ps")
                        for k in range(KD):
                            nc.tensor.matmul(ps[:], h[k][0][:, ssl], wq_h[k][:, 2 * D:3 * D],
                                             start=(k == 0), stop=False)
                        for k in range(KD):
                            nc.tensor.matmul(ps[:], h[k][0][:, ssl], wq_l[k][:, 2 * D:3 * D],
                                             start=False, stop=False)
                        for k in range(KD):
                            nc.tensor.matmul(ps[:], h[k][1][:, ssl], wq_h[k][:, 2 * D:3 * D],
                                             start=False, stop=(k == KD - 1))
                        v = v65_p.tile([128, 8 * 65], F32, tag="v65", name="v65")
                        nc.vector.memset(
                            v[:].rearrange("p (hh c) -> p hh c", hh=8)[:, :, 64:65], 1.0)
                        for hh in range(8):
                            nc.vector.tensor_add(v[:, hh * 65:hh * 65 + 64],
                                                 ps[:, hh * 64:(hh + 1) * 64],
                                                 bv_bc[:, hh * 64:(hh + 1) * 64])
                        # attention (fp32), heads software-pipelined:
                        # score(h+1) issues before attout(h) so the PE never
                        # waits on the exp/mask producer chain.
                        onat = onat_p.tile([128, D], F32, tag="onat", name="onat")
                        et_t = [None] * 8

                        def do_score(hh):
                            bp = (hh % 2) * 64
                            kt = qk[4 + hh // 2]
                            qt = qk[hh // 2]
                            s_ps = ps_at.tile([128, 128], F32, tag="at", name="sps")
                            nc.tensor.matmul(s_ps[:], kt[bp:bp + 64, tt * 128:(tt + 1) * 128],
                                             qt[bp:bp + 64, tt * 128:(tt + 1) * 128],
                                             start=True, stop=True)
                            et = e_p.tile([128, 128], F32, tag="et", name="et")
                            nc.scalar.activation(et[:], s_ps[:], AF.Exp, bias=0.0, scale=0.125)
                            nc.vector.tensor_mul(et[:], et[:], mask4_t[:, 0:128])
                            return et

                        def do_out(hh):
                            o_ps = ps_at.tile([128, 65], F32, tag="at", name="ops")
                            nc.tensor.matmul(o_ps[:], et_t[hh][:], v[:, hh * 65:(hh + 1) * 65],
                                             start=True, stop=True)
                            rcol = col2p.tile([128, 1], F32, tag="rcol", name="rcol")
                            nc.vector.reciprocal(rcol[:], o_ps[:, 64:65])
                            nc.vector.tensor_scalar_mul(onat[:, hh * 64:(hh + 1) * 64],
                                                        o_ps[:, 0:64], rcol[:])

                        for hh in range(8):
                            et_t[hh] = do_score(hh)
                            if hh >= 1:
                                do_out(hh - 1)
                        do_out(7)
                        # split onat to f16 hi/lo; defer transposes one subtile
                        on_h = onat_p.tile([128, D], F16, tag="on_h", name="on_h")
                        nc.scalar.activation(on_h[:], onat[:], AF.Identity, bias=0.0, scale=1.0)
                        on_l = onat_p.tile([128, D], F16, tag="on_l", name="on_l")
                        nc.vector.tensor_sub(on_l[:], onat[:], on_h[:])
                        if pend_tp is not None:
                            do_transposes(*pend_tp)
                        pend_tp = (tt, on_h, on_l)
                    if pend_tp is not None:
                        do_transposes(*pend_tp)
                    # Wo + residual, f16 3-pass
                    for ot in range(KD):
                        osl = slice(ot * 128, (ot + 1) * 128)
                        ps = ps_mm.tile([128, TB], F32, tag="mm", name="ps")
                        for k in range(KD):
                            nc.tensor.matmul(ps[:], wo_h[k][:, osl], oT_tiles[2 * k][:],
                                             start=(k == 0), stop=False)
                        for k in range(KD):
                            nc.tensor.matmul(ps[:], wo_h[k][:, osl], oT_tiles[2 * k + 1][:],
                                             start=False, stop=False)
                        for k in range(KD):
                            nc.tensor.matmul(ps[:], wo_l[k][:, osl], oT_tiles[2 * k][:],
                                             start=False, stop=(k == KD - 1))
                        nc.vector.scalar_tensor_tensor(xT[ot][:, tsl], ps[:],
                                                       ccol(cb + 8 + ot),
                                                       xT[ot][:, tsl], op0=ALU.add, op1=ALU.add)

                # ---- FF phase ----
                w1_h, w1_l = [], []
                for k in range(KD):
                    wh = bw_p.tile([128, FF], F16, tag="bigw", name="w1h")
                    nc.sync.dma_start(wh[:, 0:FF], w1T_h[li, k * 128:(k + 1) * 128, :])
                    w1_h.append(wh)
                for k in range(KD):
                    wl = bw_p.tile([128, FF], F16, tag="bigw", name="w1l")
                    nc.scalar.dma_start(wl[:, 0:FF], w1T_l[li, k * 128:(k + 1) * 128, :])
                    w1_l.append(wl)
                w2_h, w2_l = [], []
                for kf in range(KF):
                    wh = w2_p.tile([128, D], F16, tag="w2", name="w2h")
                    nc.sync.dma_start(wh[:], w2T_h[li, kf * 128:(kf + 1) * 128, :])
                    w2_h.append(wh)
                    wl = w2_p.tile([128, D], F16, tag="w2", name="w2l")
                    nc.scalar.dma_start(wl[:], w2T_l[li, kf * 128:(kf + 1) * 128, :])
                    w2_l.append(wl)
                # rolling stats (shares the mm PSUM ring)
                bc_cur = ln_stats(0, ps_mm, "mm")
                for b_ in range(NB):
                    tsl = slice(b_ * TB, (b_ + 1) * TB)
                    h2 = ln_h(b_, bc_cur, "h")
                    if b_ + 1 < NB:
                        bc_cur = ln_stats(b_ + 1, ps_mm, "mm")
                    acc = [ps_acc.tile([128, TB], F32, tag="acc", name="facc")[:]
                           for _ in range(KD)]
                    rl_t = [None] * KF

                    def ff1(kf):
                        fsl = slice(kf * 128, (kf + 1) * 128)
                        ps = ps_mm.tile([128, TB], F32, tag="mm", name="ps")
                        for k in range(KD):
                            nc.tensor.matmul(ps[:], w1_h[k][:, fsl], h2[k][0][:],
                                             start=(k == 0), stop=False)
                        for k in range(KD):
                            nc.tensor.matmul(ps[:], w1_h[k][:, fsl], h2[k][1][:],
                                             start=False, stop=False)
                        for k in range(KD):
                            nc.tensor.matmul(ps[:], w1_l[k][:, fsl], h2[k][0][:],
                                             start=False, stop=(k == KD - 1))
                        rl_h = relu_p.tile([128, TB], F16, tag="rl_h", name="rl_h")
                        nc.scalar.activation(rl_h[:], ps[:], AF.Relu,
                                             bias=ccol(cb + 12 + kf), scale=1.0)
                        zz = hp.tile([128, TB], F32, tag="hf", name="zz")
                        nc.vector.tensor_scalar(zz[:], ps[:], ccol(cb + 12 + kf), 0.0,
                                                op0=ALU.add, op1=ALU.max)
                        rl_l = relu_p.tile([128, TB], F16, tag="rl_l", name="rl_l")
                        nc.vector.tensor_sub(rl_l[:], zz[:], rl_h[:])
                        return (rl_h, rl_l)

                    def ff2(kf):
                        rl_h, rl_l = rl_t[kf]
                        for ot in range(KD):
                            osl = slice(ot * 128, (ot + 1) * 128)
                            nc.tensor.matmul(acc[ot], w2_h[kf][:, osl], rl_h[:],
                                             start=(kf == 0), stop=False)
                            nc.tensor.matmul(acc[ot], w2_h[kf][:, osl], rl_l[:],
                                             start=False, stop=False)
                            nc.tensor.matmul(acc[ot], w2_l[kf][:, osl], rl_h[:],
                                             start=False, stop=(kf == KF - 1))

                    for kf in range(KF):
                        rl_t[kf] = ff1(kf)
                        if kf >= 1:
                            ff2(kf - 1)
                    ff2(KF - 1)
                    for ot in range(KD):
                        nc.vector.scalar_tensor_tensor(xT[ot][:, tsl], acc[ot],
                                                       ccol(cb + 28 + ot), xT[ot][:, tsl],
                                                       op0=ALU.add, op1=ALU.add)
        # ============ generator pools released here =========================

        # ---------------- matching-stage pools (reuse generator space) ------
        outw_p = ctx.enter_context(tc.tile_pool(name="outw", bufs=1))
        outp2 = ctx.enter_context(tc.tile_pool(name="outp2", bufs=1))
        mrow = ctx.enter_context(tc.tile_pool(name="mrow", bufs=1))
        mbcp = ctx.enter_context(tc.tile_pool(name="mbc", bufs=1))
        mtch = ctx.enter_context(tc.tile_pool(name="mtch", bufs=1))
        m2p = ctx.enter_context(tc.tile_pool(name="m2p", bufs=2))
        wT_p = ctx.enter_context(tc.tile_pool(name="wTp", bufs=8))
        pnat_p = ctx.enter_context(tc.tile_pool(name="pnat", bufs=1))
        xfa_p = ctx.enter_context(tc.tile_pool(name="xfa", bufs=1))
        xfTs_p = ctx.enter_context(tc.tile_pool(name="xfTs", bufs=2))
        pts_p = ctx.enter_context(tc.tile_pool(name="pts", bufs=4))
        xsp_p = ctx.enter_context(tc.tile_pool(name="xsp", bufs=2))

        # fp32 local xf in natural layout [scene, feature]
        xfl = outp2.tile([SC, FDIM], F32, tag="xfl", name="xfl")
        # y_T [ch, tok]: fp32 (for transposes) and bf16 (matching source)
        yT_f = outp2.tile([128, T], F32, tag="yTf")
        yT_bf = outp2.tile([128, T], BF16, tag="yTbf")

        # ========= output projection (f16 3-pass from xT splits) =========
        outw_h = outw_p.tile([128, KD * CH], F16, tag="outwh", name="outwh")
        outw_l = outw_p.tile([128, KD * CH], F16, tag="outwl", name="outwl")
        for k in range(KD):
            nc.sync.dma_start(outw_h[:, k * CH:(k + 1) * CH],
                              outwT_h[k * 128:(k + 1) * 128, :])
            nc.sync.dma_start(outw_l[:, k * CH:(k + 1) * CH],
                              outwT_l[k * 128:(k + 1) * 128, :])
        outb_col = ccol(4 + 32 * DEPTH)

        for b_ in range(NB):
            bsl = slice(b_ * TB, (b_ + 1) * TB)
            xs = []
            for k in range(KD):
                xh = xsp_p.tile([128, TB], F16, tag=f"xh{k}", name="xh")
                nc.scalar.activation(xh[:], xT[k][:, bsl], AF.Identity, bias=0.0, scale=1.0)
                xl = xsp_p.tile([128, TB], F16, tag=f"xl{k}", name="xl")
                nc.vector.tensor_sub(xl[:], xT[k][:, bsl], xh[:])
                xs.append((xh, xl))
            ps = ps_mm.tile([128, TB], F32, tag="mm", name="ps")
            for k in range(KD):
                nc.tensor.matmul(ps[:], outw_h[:, k * CH:(k + 1) * CH], xs[k][0][:],
                                 start=(k == 0), stop=False)
            for k in range(KD):
                nc.tensor.matmul(ps[:], outw_h[:, k * CH:(k + 1) * CH], xs[k][1][:],
                                 start=False, stop=False)
            for k in range(KD):
                nc.tensor.matmul(ps[:], outw_l[:, k * CH:(k + 1) * CH], xs[k][0][:],
                                 start=False, stop=(k == KD - 1))
            nc.scalar.activation(yT_f[:, bsl], ps[:], AF.Identity,
                                 bias=outb_col, scale=1.0)
            nc.vector.tensor_copy(yT_bf[:, bsl], yT_f[:, bsl])

        # y natural via PE transposes of yT_f; bf16 shards into ag_in
        for tt in range(T // 128):
            tp = ps_at.tile([128, CH], F32, tag="at", name="yn_tp")
            nc.tensor.transpose(tp[:], yT_f[:, tt * 128:(tt + 1) * 128], ident[:])
            yn = m2p.tile([128, CH], F32, tag="yn", name="yn")
            nc.vector.tensor_copy(yn[:], tp[:])
            nc.sync.dma_start(
                xfl[tt * 4:(tt + 1) * 4, :].rearrange("p (l c) -> p l c", l=L),
                yn[:])
            ynbf = m2p.tile([128, CH], BF16, tag="ynbf", name="ynbf")
            nc.vector.tensor_copy(ynbf[:], yn[:])
            nc.scalar.dma_start(
                ag_in[AG_XFN + tt * 4 * FDIM: AG_XFN + (tt + 1) * 4 * FDIM]
                .rearrange("(i l c) -> i l c", l=L, c=CH),
                ynbf[:])

        # xn = ||xf_i||^2 via gram diag (bf16 inputs, fp32 accum)
        xfT_st = yT_bf[:].rearrange("c (i l) -> c l i", l=L)   # [128, 32, 64]
        gram = ps_at.tile([SC, SC], F32, tag="at", name="gram")
        for l in range(KFl):
            nc.tensor.matmul(gram[:], xfT_st[:, l, :], xfT_st[:, l, :],
                             start=(l == 0), stop=(l == KFl - 1))
        gd = m2p.tile([SC, SC], F32, tag="gd", name="gd")
        nc.vector.tensor_mul(gd[:], gram[:], ident[0:SC, 0:SC])
        xn_col = colp.tile([SC, 1], F32, tag="xncol", name="xncol")
        nc.vector.reduce_sum(xn_col[:], gd[:], axis=AX.X)
        agi_f32 = ag_in_u.bitcast(F32)
        nc.sync.dma_start(
            agi_f32[AG_XN // 2:AG_XN // 2 + SC].rearrange("(i bb) -> i bb", bb=1),
            xn_col[:])
        nc.gpsimd.collective_compute(
            "AllGather", ALU.bypass, replica_groups=[list(range(NC_))],
            ins=[ag_in_u[:]], outs=[ag_out_u[:]])

        # preload p tiles (no dependence on AG)
        pnat_t = []
        for jt in range(4):
            t = pnat_p.tile([128, FDIM], BF16, tag=f"pn{jt}", name=f"pn{jt}")
            nc.scalar.dma_start(t[:], pnat[jt * 128:(jt + 1) * 128, :])
            pnat_t.append(t)

        # S_pos (does not need AG): acc over 32 f-chunks
        spos = ps_acc.tile([SC, B], F32, tag="acc", name="spos")
        for l in range(KFl):
            mv = pts_p.tile([128, B], BF16, tag="mv", name="mv")
            nc.sync.dma_start(mv[:], pT[l * 128:(l + 1) * 128, :])
            nc.tensor.matmul(spos[:], xfT_st[:, l, :], mv[:],
                             start=(l == 0), stop=(l == KFl - 1))

        # xn_full row [1, 512] f32 + broadcast
        ago_f32 = ag_out_u.bitcast(F32)
        xn_row = mrow.tile([1, B], F32, tag="mr", name="xnrow")
        nc.sync.dma_start(
            xn_row[:],
            bass.AP(tensor=ago_f32.tensor, offset=ago_f32.offset + AG_XN // 2,
                    ap=[[1, 1], [AG_SZ // 2, NC_], [1, SC]]))
        xn_bc = mbcp.tile([SC, B], F32, tag="mbc", name="xnbc")
        nc.gpsimd.partition_broadcast(xn_bc[:], xn_row[:])

        # xf_nat_all: 4 scene-tiles [128, 4096] bf16 (8KB lines)
        xfa = []
        for st in range(4):
            t = xfa_p.tile([128, FDIM], BF16, tag=f"xfa{st}", name=f"xfa{st}")
            for half in range(2):
                c = 2 * st + half
                nc.sync.dma_start(
                    t[half * SC:(half + 1) * SC, :],
                    bass.AP(tensor=ag_out.tensor,
                            offset=ag_out.offset + c * AG_SZ + AG_XFN,
                            ap=[[FDIM, SC], [1, FDIM]]))
            xfa.append(t)

        # S_neg: rebuild xf^T_all [128 f, 512 scene] per l-chunk via PE
        # transposes (double-buffered), accumulate immediately
        sneg = ps_acc.tile([SC, B], F32, tag="acc", name="sneg")
        for l in range(KFl):
            xfT_l = xfTs_p.tile([128, B], BF16, tag="xfTs", name="xfTs")
            for st in range(4):
                tp = ps_at.tile([128, 128], BF16, tag="at", name="ttp")
                nc.tensor.transpose(tp[:], xfa[st][:, l * 128:(l + 1) * 128],
                                    ident_bf[:])
                nc.vector.tensor_copy(xfT_l[:, st * 128:(st + 1) * 128], tp[:])
            nc.tensor.matmul(sneg[:], xfT_st[:, l, :], xfT_l[:],
                             start=(l == 0), stop=(l == KFl - 1))

        # distances -> logits -> E (in place); pos half first (no AG dep)
        dist = mtch.tile([SC, 2 * B], F32, tag="dist")
        nc.vector.scalar_tensor_tensor(dist[:, 0:B], spos[:], -2.0, pn_t[:],
                                       op0=ALU.mult, op1=ALU.add)
        nc.vector.tensor_scalar_add(dist[:, 0:B], dist[:, 0:B], xn_col[:])
        nc.vector.tensor_scalar_max(dist[:, 0:B], dist[:, 0:B], 0.0)
        nc.scalar.activation(dist[:, 0:B], dist[:, 0:B], AF.Sqrt, bias=0.0, scale=1.0)
        nc.vector.scalar_tensor_tensor(dist[:, B:2 * B], sneg[:], -2.0, xn_bc[:],
                                       op0=ALU.mult, op1=ALU.add)
        nc.vector.tensor_scalar_add(dist[:, B:2 * B], dist[:, B:2 * B], xn_col[:])
        nc.vector.tensor_scalar_max(dist[:, B:2 * B], dist[:, B:2 * B], 0.0)
        nc.scalar.activation(dist[:, B:2 * B], dist[:, B:2 * B], AF.Sqrt, bias=0.0, scale=1.0)
        nc.vector.tensor_add(dist[:, B:2 * B], dist[:, B:2 * B], nd_t[:])
        dmin = colp.tile([SC, 1], F32, tag="dmin", name="dmin")
        nc.vector.tensor_reduce(out=dmin[:], in_=dist[:], axis=AX.X, op=ALU.min)
        E = dist  # in place: E = exp(-d + dmin)
        nc.scalar.activation(E[:], dist[:], AF.Exp, bias=dmin[:], scale=-1.0)
        g_col = colp.tile([SC, 1], F32, tag="gcol", name="gcol")
        nc.scalar.activation(g_col[:], dmin[:], AF.Exp, bias=m20_col[:], scale=-1.0)
        sr_col = colp.tile([SC, 1], F32, tag="srcol", name="srcol")
        nc.vector.reduce_sum(sr_col[:], E[:], axis=AX.X)
        # partial colsums of G = E * g_i via g-weighted stationary
        cs_row = mrow.tile([1, 2 * B], F32, tag="mr", name="csrow")
        for b_ in range(2):
            ps = ps_mm.tile([1, B], F32, tag="mm", name="ps")
            nc.tensor.matmul(ps[:], g_col[:], E[:, b_ * B:(b_ + 1) * B],
                             start=True, stop=True)
            nc.vector.tensor_copy(cs_row[:, b_ * B:(b_ + 1) * B], ps[:])
        nc.sync.dma_start(ar_in, cs_row[:])
        nc.gpsimd.collective_compute(
            "AllReduce", ALU.add, replica_groups=[list(range(NC_))],
            ins=[ar_in[:]], outs=[ar_out[:]])
        cs_g = mrow.tile([1, 2 * B], F32, tag="mr", name="csg")
        nc.sync.dma_start(cs_g[:], ar_out)
        cs_bc = mbcp.tile([SC, 2 * B], F32, tag="csbc", name="csbc")
        nc.gpsimd.partition_broadcast(cs_bc[:], cs_g[:])
        nc.scalar.activation(cs_bc[:], cs_bc[:], AF.Sqrt, bias=0.0, scale=1.0)
        nc.vector.reciprocal(cs_bc[:], cs_bc[:])
        # E' = E * invsqrt(Sc); row scalars BEFORE overwriting E with W
        nc.vector.tensor_mul(E[:], E[:], cs_bc[:])
        snp = colp.tile([SC, 1], F32, tag="snp", name="snp")
        nc.vector.reduce_sum(snp[:], E[:, B:2 * B], axis=AX.X)
        spp = colp.tile([SC, 1], F32, tag="spp", name="spp")
        nc.vector.reduce_sum(spp[:], E[:, 0:B], axis=AX.X)
        tcol = colp.tile([SC, 1], F32, tag="tcol", name="tcol")
        nc.vector.reciprocal(tcol[:], sr_col[:])
        nc.vector.tensor_mul(tcol[:], tcol[:], g_col[:])
        ccl = colp.tile([SC, 1], F32, tag="ccol", name="ccol")
        nc.scalar.activation(ccl[:], tcol[:], AF.Sqrt, bias=0.0, scale=1.0)
        alpha = colp.tile([SC, 1], F32, tag="alpha", name="alpha")
        nc.vector.tensor_mul(alpha[:], tcol[:], snp[:])
        beta = colp.tile([SC, 1], F32, tag="beta", name="beta")
        nc.vector.tensor_mul(beta[:], alpha[:], spp[:])
        nc.vector.tensor_mul(beta[:], beta[:], ccl[:])
        nc.vector.tensor_scalar_mul(beta[:], beta[:], -1.0)
        # W = E' * alpha / -beta (in place), transpose, cast bf16
        nc.vector.tensor_scalar_mul(E[:, 0:B], E[:, 0:B], alpha[:])
        nc.vector.tensor_scalar_mul(E[:, B:2 * B], E[:, B:2 * B], beta[:])
        wT = []
        for half in range(2):
            for jt in range(4):
                tp = ps_at.tile([128, SC], F32, tag="at", name="wtp")
                nc.tensor.transpose(
                    tp[:], E[:, half * B + jt * 128: half * B + (jt + 1) * 128],
                    ident[0:SC, 0:SC])
                t = wT_p.tile([128, SC], BF16, tag="wT", name="wT")
                nc.vector.tensor_copy(t[:], tp[:])
                wT.append(t)
        # V and loss: V = Wpos @ p - Wneg @ xf_full, r = xf - fl(xf + V)
        # everything SBUF-resident
        lacc = m2p.tile([SC, 16], F32, tag="lacc", name="lacc", bufs=1)
        FBW = 256
        for fb in range(FDIM // FBW):
            fsl = slice(fb * FBW, (fb + 1) * FBW)
            vps = ps_acc.tile([SC, FBW], F32, tag="acc", name="vps")
            for jt in range(4):
                nc.tensor.matmul(vps[:], wT[jt][:], pnat_t[jt][:, fsl],
                                 start=(jt == 0), stop=False)
            for jt in range(4):
                nc.tensor.matmul(vps[:], wT[4 + jt][:], xfa[jt][:, fsl],
                                 start=False, stop=(jt == 3))
            t1 = m2p.tile([SC, FBW], F32, tag="t1", name="t1")
            nc.vector.tensor_add(t1[:], xfl[:, fsl], vps[:])
            nc.vector.tensor_sub(t1[:], xfl[:, fsl], t1[:])
            nc.vector.tensor_mul(t1[:], t1[:], t1[:])
            nc.vector.reduce_sum(lacc[:, fb:fb + 1], t1[:], axis=AX.X)
        lsum = colp.tile([SC, 1], F32, tag="lsum", name="lsum")
        nc.vector.reduce_sum(lsum[:], lacc[:], axis=AX.X)
        tot = ps_mm.tile([1, 1], F32, tag="mm", name="tot")
        nc.tensor.matmul(tot[:], ones_col[0:SC, :], lsum[:], start=True, stop=True)
        tot_sb = colp.tile([1, 1], F32, tag="tot", name="totsb")
        nc.vector.tensor_copy(tot_sb[:], tot[:])
        nc.sync.dma_start(loss_part, tot_sb[:])

    nc.compile()
    return nc


_NC_CACHE = None


def _get_nc():
    global _NC_CACHE
    if _NC_CACHE is None:
        _NC_CACHE = _build_nc()
    return _NC_CACHE


def _split16(a):
    """f16 hi/lo split (round-to-nearest): a ~= hi + lo."""
    hi = a.astype(np.float16)
    lo = (a - hi.astype(np.float32)).astype(np.float16)
    return np.ascontiguousarray(hi), np.ascontiguousarray(lo)


def _prep_inputs(inputs):
    f32 = lambda x: np.ascontiguousarray(np.asarray(x), dtype=np.float32)
    bf = lambda x: np.ascontiguousarray(np.asarray(x, dtype=ml_dtypes.bfloat16))
    sample_p = f32(inputs["sample_p"])
    eps = f32(inputs["eps"])
    p2 = sample_p.reshape(B, FDIM)
    pn = (p2.astype(np.float64) ** 2).sum(-1).astype(np.float32)

    g1 = f32(inputs["ln1_g"])   # [DEPTH, D]
    b1n = f32(inputs["ln1_b"])
    g2 = f32(inputs["ln2_g"])
    b2n = f32(inputs["ln2_b"])
    Wqkv = f32(inputs["Wqkv"])  # [DEPTH, 3D, D]
    W1 = f32(inputs["W1"])      # [DEPTH, FF, D]

    # fold LN gamma into weights, LN beta into biases
    Gqkv = Wqkv * g1[:, None, :]
    bqkv_eff = f32(inputs["bqkv"]) + np.einsum('dij,dj->di', Wqkv, b1n)
    G1 = W1 * g2[:, None, :]
    b1_eff = f32(inputs["b1"]) + np.einsum('dij,dj->di', W1, b2n)

    # packed bias columns [128, NCOLS]
    colsP = np.zeros((128, NCOLS), np.float32)
    inb = f32(inputs["in_b"])
    for k in range(KD):
        colsP[:, k] = inb[k * 128:(k + 1) * 128]
    for li in range(DEPTH):
        cb = 4 + 32 * li
        for ot in range(8):
            colsP[:, cb + ot] = bqkv_eff[li, ot * 128:(ot + 1) * 128]
        for k in range(KD):
            colsP[:, cb + 8 + k] = f32(inputs["bo"])[li, k * 128:(k + 1) * 128]
            colsP[:, cb + 28 + k] = f32(inputs["b2"])[li, k * 128:(k + 1) * 128]
        for kf in range(KF):
            colsP[:, cb + 12 + kf] = b1_eff[li, kf * 128:(kf + 1) * 128]
    colsP[:, 4 + 32 * DEPTH] = f32(inputs["out_b"])

    inwT_h, inwT_l = _split16(f32(inputs["in_w"]).T.copy())
    wqkvT_h, wqkvT_l = _split16(np.ascontiguousarray(Gqkv.transpose(0, 2, 1)))
    woT_h, woT_l = _split16(np.ascontiguousarray(f32(inputs["Wo"]).transpose(0, 2, 1)))
    w1T_h, w1T_l = _split16(np.ascontiguousarray(G1.transpose(0, 2, 1)))
    w2T_h, w2T_l = _split16(np.ascontiguousarray(f32(inputs["W2"]).transpose(0, 2, 1)))
    outwT_h, outwT_l = _split16(f32(inputs["out_w"]).T.copy())

    common = {
        "inwT_h": inwT_h, "inwT_l": inwT_l,
        "wqkvT_h": wqkvT_h, "wqkvT_l": wqkvT_l,
        "bqkv": bqkv_eff,
        "woT_h": woT_h, "woT_l": woT_l,
        "w1T_h": w1T_h, "w1T_l": w1T_l,
        "w2T_h": w2T_h, "w2T_l": w2T_l,
        "outwT_h": outwT_h, "outwT_l": outwT_l,
        "colsP": colsP,
        "pT": bf(p2.T),
        "pnat": bf(p2),
        "pn_bc": np.broadcast_to(pn[None, :], (SC, B)).copy(),
        "attn_mask": np.tile(np.kron(np.eye(4, dtype=np.float32), np.ones((32, 32), np.float32)), (1, 4)),
    }
    in_maps = []
    for c in range(NC_):
        nd = np.zeros((SC, B), np.float32)
        nd[np.arange(SC), SC * c + np.arange(SC)] = 1e6
        m = dict(common)
        eT = eps[c * SC:(c + 1) * SC].reshape(T, CH).T.copy()
        eh, el = _split16(eT)
        m["epsT_h"] = eh
        m["epsT_l"] = el
        m["negdiag"] = nd
        in_maps.append(m)
    return in_maps


def kernel(**inputs) -> np.ndarray:
    nc = _get_nc()
    in_maps = _prep_inputs(inputs)
    res = run_bass_kernel_spmd(nc, in_maps, list(range(NC_)))
    total = sum(float(r["loss_part"][0, 0]) for r in res.results)
    return np.float32(total / (B * FDIM))


# revision 19
# speedup vs baseline: 1.2316x; 1.0176x over previous
"""Trainium2 Bass kernel for nn_DriftScene_88270167868070.

Contract: kernel(**inputs) takes FULL unsharded inputs (as produced by
setup_inputs()) and returns the FULL output (a scalar np.float32).

Strategy (8 NeuronCores, one SPMD launch):
  - Data-parallel transformer generator over the batch (64 scenes/core).
  - Big GEMMs (in-proj, QKV, V, Wo, FF1, FF2, out-proj) run as f16
    3-pass hi/lo splits: C = Ah.Wh + Al.Wh + Ah.Wl.  Each pass streams at
    1 cyc/row (vs fp32's 4), giving ~fp32-equivalent precision (~22 bits)
    at 3/4 the PE cycles; validated vs the jax fp32 reference at ~1e-3.
  - LN gamma/beta are folded into the weights/biases on the host, so the
    kernel normalizes to h0=(x-m)*rstd only.  LN variance uses an f16
    round-to-nearest x^2 (unbiased); the mean-sum stays fp32.
  - Attention (scores, softmax, attn.v) stays fp32; attention outputs are
    split to f16 and transposed on the PE in f16 (1 cyc/row).
  - Matching stage row-sharded, bf16, fully SBUF-resident (AllGather of
    bf16 xf + norms; column-softmax via AllReduce).
  - loss = mean((xf - fl32(xf + V))^2) with explicit fp32 rounding.
"""

import numpy as np
from contextlib import ExitStack

import concourse.bass as bass
import concourse.tile as tile
from concourse import bacc, mybir
from concourse.bass_utils import run_bass_kernel_spmd
from concourse.masks import make_identity
import ml_dtypes

F32 = mybir.dt.float32
F16 = mybir.dt.float16
BF16 = mybir.dt.bfloat16
AF = mybir.ActivationFunctionType
ALU = mybir.AluOpType
AX = mybir.AxisListType

# Problem dims (hardcoded per contract)
B, L, CH = 512, 32, 128
D, HEADS, DEPTH, FF = 512, 8, 4, 2048
DH = D // HEADS
LN_EPS = 1e-5
NC_ = 8                 # cores
SC = B // NC_           # 64 scenes per core
T = SC * L              # 2048 tokens per core
TB = 512                # tokens per t-block
NB = T // TB            # 4 t-blocks
NS = TB // 128          # 4 subtiles per block
KD = D // 128           # 4 d-tiles
KF = FF // 128          # 16 ff-tiles
FDIM = L * CH           # 4096 flattened feature dim
KFl = FDIM // 128       # 32 f-tiles
M_SHIFT = -20.0         # global shift for column softmax stabilization

# packed AllGather layout (bf16 element offsets): xf_nat + xn bits
AG_XFN = 0                      # xf_nat [64, 4096]
AG_XN = FDIM * SC               # xn bits: f32 [64,1] viewed as bf16 [64,2]
AG_SZ = FDIM * SC + 2 * SC      # 262272

# packed bias-column layout: [128, NCOLS] host-prepped
# idx 0..3: in_b chunks; per layer li base 4+32*li:
#   +0..7 bqkv_eff[0:1024] (Q,K), +8..11 bo, +12..27 b1_eff, +28..31 b2
# idx 4+32*DEPTH: out_b
NCOLS = 4 + 32 * DEPTH + 1


def _build_nc():
    nc = bacc.Bacc("TRN2", target_bir_lowering=False, debug=False, num_devices=NC_)

    # ---------------- I/O ----------------
    def inp(name, shape, dt=F32):
        return nc.dram_tensor(name, shape, dt, kind="ExternalInput").ap()

    epsT_h = inp("epsT_h", [128, T], F16)     # eps shard hi, [ch, tok]
    epsT_l = inp("epsT_l", [128, T], F16)
    inwT_h = inp("inwT_h", [128, D], F16)     # in_w.T hi/lo
    inwT_l = inp("inwT_l", [128, D], F16)
    wqkvT_h = inp("wqkvT_h", [DEPTH, D, 3 * D], F16)  # (Wqkv*g).T hi/lo
    wqkvT_l = inp("wqkvT_l", [DEPTH, D, 3 * D], F16)
    bqkv = inp("bqkv", [DEPTH, 3 * D])        # effective qkv bias
    woT_h = inp("woT_h", [DEPTH, D, D], F16)
    woT_l = inp("woT_l", [DEPTH, D, D], F16)
    w1T_h = inp("w1T_h", [DEPTH, D, FF], F16)  # (W1*g2).T hi/lo
    w1T_l = inp("w1T_l", [DEPTH, D, FF], F16)
    w2T_h = inp("w2T_h", [DEPTH, FF, D], F16)
    w2T_l = inp("w2T_l", [DEPTH, FF, D], F16)
    outwT_h = inp("outwT_h", [D, CH], F16)
    outwT_l = inp("outwT_l", [D, CH], F16)
    colsP = inp("colsP", [128, NCOLS])        # packed bias columns
    pT = inp("pT", [FDIM, B], BF16)           # sample_p transposed [f, scene]
    pnat = inp("pnat", [B, FDIM], BF16)       # sample_p natural
    pn_bc = inp("pn_bc", [SC, B])             # ||p_j||^2 broadcast rows
    attn_mask = inp("attn_mask", [128, 512])  # 4-scene block-diag 0/1, x4 heads
    negdiag = inp("negdiag", [SC, B])         # 1e6 at (i, SC*core + i)

    loss_part = nc.dram_tensor("loss_part", [1, 1], F32, kind="ExternalOutput").ap()

    # ---------------- DRAM scratch ----------------
    ag_in_u = nc.dram_tensor("ag_in", [AG_SZ], mybir.dt.uint16).ap()
    ag_out_u = nc.dram_tensor("ag_out", [NC_ * AG_SZ], mybir.dt.uint16,
                              addr_space="Shared").ap()
    ag_in = ag_in_u.bitcast(BF16)
    ag_out = ag_out_u.bitcast(BF16)
    ar_in = nc.dram_tensor("ar_in", [1, 2 * B], F32).ap()
    ar_out = nc.dram_tensor("ar_out", [1, 2 * B], F32, addr_space="Shared").ap()

    with tile.TileContext(nc) as tc, ExitStack() as ctx:
        # ---------------- long-lived pools (bufs is PER TAG) ----------------
        const = ctx.enter_context(tc.tile_pool(name="const", bufs=1))
        xTp = ctx.enter_context(tc.tile_pool(name="xT", bufs=1))
        rowp = ctx.enter_context(tc.tile_pool(name="rows", bufs=3))
        bcp = ctx.enter_context(tc.tile_pool(name="bc", bufs=2))
        colp = ctx.enter_context(tc.tile_pool(name="colp", bufs=1))
        col2p = ctx.enter_context(tc.tile_pool(name="col2p", bufs=4))

        ps_mm = ctx.enter_context(tc.tile_pool(name="ps_mm", bufs=2, space="PSUM"))
        ps_acc = ctx.enter_context(tc.tile_pool(name="ps_acc", bufs=4, space="PSUM"))
        ps_at = ctx.enter_context(tc.tile_pool(name="ps_at", bufs=2, space="PSUM"))

        # ---------------- constants ----------------
        ident = const.tile([128, 128], F32)
        make_identity(nc, ident[:])
        ident_bf = const.tile([128, 128], BF16)
        nc.vector.tensor_copy(ident_bf[:], ident[:])
        ident16 = const.tile([128, 128], F16)
        nc.vector.tensor_copy(ident16[:], ident[:])
        ones_col = const.tile([128, 1], F32)
        nc.vector.memset(ones_col[:], 1.0)
        ones16 = const.tile([128, 1], F16)
        nc.vector.memset(ones16[:], 1.0)
        mask4_t = const.tile([128, 512], F32)
        nc.sync.dma_start(mask4_t[:], attn_mask)
        pn_t = const.tile([SC, B], F32)
        nc.sync.dma_start(pn_t[:], pn_bc)
        nd_t = const.tile([SC, B], F32)
        nc.sync.dma_start(nd_t[:], negdiag)
        eps_col = const.tile([1, 1], F32)
        nc.vector.memset(eps_col[:], LN_EPS)
        m20_col = const.tile([SC, 1], F32)
        nc.vector.memset(m20_col[:], -M_SHIFT)
        cols = const.tile([128, NCOLS], F32)
        nc.sync.dma_start(cols[:], colsP)

        def ccol(idx):
            return cols[:, idx:idx + 1]

        # residual stream X_T: KD tiles [128, T] fp32, persistent
        xT = [xTp.tile([128, T], F32, tag=f"xT{k}", name=f"xT{k}") for k in range(KD)]

        # ============ generator scope (pools released before matching) ======
        with ExitStack() as gctx:
            hp = gctx.enter_context(tc.tile_pool(name="h", bufs=2))
            sqp = gctx.enter_context(tc.tile_pool(name="sq", bufs=2))
            bw_p = gctx.enter_context(tc.tile_pool(name="bigw", bufs=8))
            wo_p = gctx.enter_context(tc.tile_pool(name="wo", bufs=8))
            w2_p = gctx.enter_context(tc.tile_pool(name="w2", bufs=32))
            qk_p = gctx.enter_context(tc.tile_pool(name="qk", bufs=8))
            v65_p = gctx.enter_context(tc.tile_pool(name="v65", bufs=2))
            e_p = gctx.enter_context(tc.tile_pool(name="et", bufs=3))
            onat_p = gctx.enter_context(tc.tile_pool(name="onat", bufs=2))
            oT_p = gctx.enter_context(tc.tile_pool(name="oT", bufs=1))
            relu_p = gctx.enter_context(tc.tile_pool(name="relu", bufs=2))

            # ====== input projection: X_T = (eps @ in_w.T).T, f16 3-pass ====
            inw_h = bw_p.tile([128, FF], F16, tag="bigw", name="inw_h")
            inw_l = bw_p.tile([128, FF], F16, tag="bigw", name="inw_l")
            nc.sync.dma_start(inw_h[:, 0:D], inwT_h)
            nc.sync.dma_start(inw_l[:, 0:D], inwT_l)
            for b_ in range(NB):
                bsl = slice(b_ * TB, (b_ + 1) * TB)
                eps_h = sqp.tile([128, TB], F16, tag="sq", name="eps_h")
                eps_l = sqp.tile([128, TB], F16, tag="sq", name="eps_l")
                nc.sync.dma_start(eps_h[:], epsT_h[:, bsl])
                nc.sync.dma_start(eps_l[:], epsT_l[:, bsl])
                for dt_ in range(KD):
                    dsl = slice(dt_ * 128, (dt_ + 1) * 128)
                    ps = ps_mm.tile([128, TB], F32, tag="mm", name="ps")
                    nc.tensor.matmul(ps[:], inw_h[:, dsl], eps_h[:], start=True, stop=False)
                    nc.tensor.matmul(ps[:], inw_h[:, dsl], eps_l[:], start=False, stop=False)
                    nc.tensor.matmul(ps[:], inw_l[:, dsl], eps_h[:], start=False, stop=True)
                    nc.scalar.activation(xT[dt_][:, bsl], ps[:],
                                         AF.Identity, bias=ccol(dt_), scale=1.0)

            # ========= LN stats wave: rstd/shift broadcasts per block =========
            def ln_stats(b_, ps_pool, ps_tag):
                """PE stat sums + row chain + broadcasts for tokens of block
                b_.  Returns [128, 2*TB] tile: [:, :TB]=rstd, [:, TB:]=shift."""
                bsl = slice(b_ * TB, (b_ + 1) * TB)
                s_row = rowp.tile([1, TB], F32, tag="srow", name="srow")[:]
                q_row = rowp.tile([1, TB], F32, tag="qrow", name="qrow")[:]
                msq = rowp.tile([1, TB], F32, tag="msq", name="msq")[:]
                ps_s = ps_pool.tile([1, TB], F32, tag=ps_tag, name="ps_s")
                for k in range(KD):
                    nc.tensor.matmul(ps_s[:], ones_col[:], xT[k][:, bsl],
                                     start=(k == 0), stop=(k == KD - 1))
                nc.vector.tensor_scalar_mul(s_row, ps_s[:], -1.0 / D)   # -mean
                ps_q = ps_pool.tile([1, TB], F32, tag=ps_tag, name="ps_q")
                for k in range(KD):
                    sq = sqp.tile([128, TB], F16, tag="sq", name="sq")
                    nc.vector.tensor_mul(sq[:], xT[k][:, bsl], xT[k][:, bsl])
                    nc.tensor.matmul(ps_q[:], ones16[:], sq[:],
                                     start=(k == 0), stop=(k == KD - 1))
                nc.vector.tensor_mul(msq, s_row, s_row)
                # var = q/D - m^2  (into q_row)
                nc.vector.scalar_tensor_tensor(q_row, ps_q[:], 1.0 / D, msq,
                                               op0=ALU.mult, op1=ALU.subtract)
                # rstd = 1/sqrt(var + eps): sqrt into msq, recip into q_row
                nc.scalar.activation(msq, q_row, AF.Sqrt, bias=eps_col[:], scale=1.0)
                nc.vector.reciprocal(q_row, msq)
                # shift = -m * rstd (into s_row)
                nc.vector.tensor_mul(s_row, s_row, q_row)
                bc = bcp.tile([128, 2 * TB], F32, tag="lnbc", name="lnbc")
                nc.gpsimd.partition_broadcast(bc[:, 0:TB], q_row)
                nc.gpsimd.partition_broadcast(bc[:, TB:2 * TB], s_row)
                return bc

            def ln_h(b_, bc, htag):
                """h0 = (x - m)*rstd f16 hi/lo tile pairs for block b_."""
                bsl = slice(b_ * TB, (b_ + 1) * TB)
                hs = []
                for k in range(KD):
                    hf = hp.tile([128, TB], F32, tag="hf", name=f"{htag}f")
                    nc.vector.tensor_mul(hf[:], xT[k][:, bsl], bc[:, 0:TB])
                    nc.vector.tensor_add(hf[:], hf[:], bc[:, TB:2 * TB])
                    hh = hp.tile([128, TB], F16, tag=f"{htag}h{k}", name=f"{htag}h")
                    nc.scalar.activation(hh[:], hf[:], AF.Identity, bias=0.0, scale=1.0)
                    hl = hp.tile([128, TB], F16, tag=f"{htag}l{k}", name=f"{htag}l")
                    nc.vector.tensor_sub(hl[:], hf[:], hh[:])
                    hs.append((hh, hl))
                return hs

            # ========= transformer layers =========
            for li in range(DEPTH):
                cb = 4 + 32 * li  # packed-column base for this layer
                # ---- attention phase ----
                wq_h, wq_l = [], []
                for k in range(KD):
                    wh = bw_p.tile([128, FF], F16, tag="bigw", name="wqh")
                    nc.sync.dma_start(wh[:, 0:3 * D], wqkvT_h[li, k * 128:(k + 1) * 128, :])
                    wq_h.append(wh)
                    wl = bw_p.tile([128, FF], F16, tag="bigw", name="wql")
                    nc.scalar.dma_start(wl[:, 0:3 * D], wqkvT_l[li, k * 128:(k + 1) * 128, :])
                    wq_l.append(wl)
                wo_h, wo_l = [], []
                for k in range(KD):
                    wh = wo_p.tile([128, D], F16, tag="wo", name="woh")
                    nc.sync.dma_start(wh[:], woT_h[li, k * 128:(k + 1) * 128, :])
                    wo_h.append(wh)
                    wl = wo_p.tile([128, D], F16, tag="wo", name="wol")
                    nc.scalar.dma_start(wl[:], woT_l[li, k * 128:(k + 1) * 128, :])
                    wo_l.append(wl)
                bv_bc = bcp.tile([128, D], F32, tag="bvbc", name="bvbc", bufs=1)
                nc.gpsimd.dma_start(bv_bc[:], bass.AP(
                    tensor=bqkv.tensor, offset=bqkv.offset + li * 3 * D + 2 * D,
                    ap=[[0, 128], [1, D]]))

                # rolling stats: block b+1's stats issue early in block b
                bc_cur = ln_stats(0, ps_acc, "acc")
                for b_ in range(NB):
                    tsl = slice(b_ * TB, (b_ + 1) * TB)
                    h = ln_h(b_, bc_cur, "h")
                    if b_ + 1 < NB:
                        bc_cur = ln_stats(b_ + 1, ps_acc, "acc")
                    # oT tiles for this block: [2k]=hi, [2k+1]=lo per d-tile
                    oT_tiles = [oT_p.tile([128, TB], F16, tag=f"oT{j}", name="oT")
                                for j in range(2 * KD)]
                    # Q,K projections (transposed out), f16 3-pass
                    qk = []
                    for ot in range(8):
                        osl = slice(ot * 128, (ot + 1) * 128)
                        ps = ps_mm.tile([128, TB], F32, tag="mm", name="ps")
                        for k in range(KD):
                            nc.tensor.matmul(ps[:], wq_h[k][:, osl], h[k][0][:],
                                             start=(k == 0), stop=False)
                        for k in range(KD):
                            nc.tensor.matmul(ps[:], wq_h[k][:, osl], h[k][1][:],
                                             start=False, stop=False)
                        for k in range(KD):
                            nc.tensor.matmul(ps[:], wq_l[k][:, osl], h[k][0][:],
                                             start=False, stop=(k == KD - 1))
                        t = qk_p.tile([128, TB], F32, tag="qk", name="qk")
                        nc.scalar.activation(t[:], ps[:], AF.Identity,
                                             bias=ccol(cb + ot), scale=1.0)
                        qk.append(t)
                    def do_transposes(tt_, onh_, onl_):
                        for k in range(KD):
                            ksl = slice(k * 128, (k + 1) * 128)
                            tp = ps_at.tile([128, 128], F16, tag="at", name="tp")
                            nc.tensor.transpose(tp[:], onh_[:, ksl], ident16[:])
                            nc.vector.tensor_copy(
                                oT_tiles[2 * k][:, tt_ * 128:(tt_ + 1) * 128], tp[:])
                            tp2 = ps_at.tile([128, 128], F16, tag="at", name="tp2")
                            nc.tensor.transpose(tp2[:], onl_[:, ksl], ident16[:])
                            nc.vector.tensor_copy(
                                oT_tiles[2 * k + 1][:, tt_ * 128:(tt_ + 1) * 128], tp2[:])

                    pend_tp = None
                    for tt in range(NS):
                        ssl = slice(tt * 128, (tt + 1) * 128)
                        # V natural for this subtile, 65-strided with ones column
                        ps = ps_mm.tile([128, D], F32, tag="mm", name="ps")
                        for k in range(KD):
                            nc.tensor.matmul(ps[:], h[k][0][:, ssl], wq_h[k][:, 2 * D:3 * D],
                                             start=(k == 0), stop=False)
                        for k in range(KD):
                            nc.tensor.matmul(ps[:], h[k][0][:, ssl], wq_l[k][:, 2 * D:3 * D],
                                             start=False, stop=False)
                        for k in range(KD):
                            nc.tensor.matmul(ps[:], h[k][1][:, ssl], wq_h[k][:, 2 * D:3 * D],
                                             start=False, stop=(k == KD - 1))
                        v = v65_p.tile([128, 8 * 65], F32, tag="v65", name="v65")
                        nc.vector.memset(
                            v[:].rearrange("p (hh c) -> p hh c", hh=8)[:, :, 64:65], 1.0)
                        for hh in range(8):
                            nc.vector.tensor_add(v[:, hh * 65:hh * 65 + 64],
                                                 ps[:, hh * 64:(hh + 1) * 64],
                                                 bv_bc[:, hh * 64:(hh + 1) * 64])
                        # attention (fp32), heads software-pipelined:
                        # score(h+1) issues before attout(h) so the PE never
                        # waits on the exp/mask producer chain.
                        onat = onat_p.tile([128, D], F32, tag="onat", name="onat")
                        et_t = [None] * 8

                        def do_score(hh):
                            bp = (hh % 2) * 64
                            kt = qk[4 + hh // 2]
                            qt = qk[hh // 2]
                            s_ps = ps_at.tile([128, 128], F32, tag="at", name="sps")
                            nc.tensor.matmul(s_ps[:], kt[bp:bp + 64, tt * 128:(tt + 1) * 128],
                                             qt[bp:bp + 64, tt * 128:(tt + 1) * 128],
                                             start=True, stop=True)
                            et = e_p.tile([128, 128], F32, tag="et", name="et")
                            nc.scalar.activation(et[:], s_ps[:], AF.Exp, bias=0.0, scale=0.125)
                            nc.vector.tensor_mul(et[:], et[:], mask4_t[:, 0:128])
                            return et

                        def do_out(hh):
                            o_ps = ps_at.tile([128, 65], F32, tag="at", name="ops")
                            nc.tensor.matmul(o_ps[:], et_t[hh][:], v[:, hh * 65:(hh + 1) * 65],
                                             start=True, stop=True)
                            rcol = col2p.tile([128, 1], F32, tag="rcol", name="rcol")
                            nc.vector.reciprocal(rcol[:], o_ps[:, 64:65])
                            nc.vector.tensor_scalar_mul(onat[:, hh * 64:(hh + 1) * 64],
                                                        o_ps[:, 0:64], rcol[:])

                        for hh in range(8):
                            et_t[hh] = do_score(hh)
                            if hh >= 1:
                                do_out(hh - 1)
                        do_out(7)
                        # split onat to f16 hi/lo; defer transposes one subtile
                        on_h = onat_p.tile([128, D], F16, tag="on_h", name="on_h")
                        nc.scalar.activation(on_h[:], onat[:], AF.Identity, bias=0.0, scale=1.0)
                        on_l = onat_p.tile([128, D], F16, tag="on_l", name="on_l")
                        nc.vector.tensor_sub(on_l[:], onat[:], on_h[:])
                        if pend_tp is not None:
                            do_transposes(*pend_tp)
                        pend_tp = (tt, on_h, on_l)
                    if pend_tp is not None:
                        do_transposes(*pend_tp)
                    # Wo + residual, f16 3-pass
                    for ot in range(KD):
                        osl = slice(ot * 128, (ot + 1) * 128)
                        ps = ps_mm.tile([128, TB], F32, tag="mm", name="ps")
                        for k in range(KD):
                            nc.tensor.matmul(ps[:], wo_h[k][:, osl], oT_tiles[2 * k][:],
                                             start=(k == 0), stop=False)
                        for k in range(KD):
                            nc.tensor.matmul(ps[:], wo_h[k][:, osl], oT_tiles[2 * k + 1][:],
                                             start=False, stop=False)
                        for k in range(KD):
                            nc.tensor.matmul(ps[:], wo_l[k][:, osl], oT_tiles[2 * k][:],
                                             start=False, stop=(k == KD - 1))
                        nc.vector.scalar_tensor_tensor(xT[ot][:, tsl], ps[:],
                                                       ccol(cb + 8 + ot),
                                                       xT[ot][:, tsl], op0=ALU.add, op1=ALU.add)

                # ---- FF phase ----
                w1_h, w1_l = [], []
                for k in range(KD):
                    wh = bw_p.tile([128, FF], F16, tag="bigw", name="w1h")
                    nc.sync.dma_start(wh[:, 0:FF], w1T_h[li, k * 128:(k + 1) * 128, :])
                    w1_h.append(wh)
                for k in range(KD):
                    wl = bw_p.tile([128, FF], F16, tag="bigw", name="w1l")
                    nc.scalar.dma_start(wl[:, 0:FF], w1T_l[li, k * 128:(k + 1) * 128, :])
                    w1_l.append(wl)
                w2_h, w2_l = [], []
                for kf in range(KF):
                    wh = w2_p.tile([128, D], F16, tag="w2", name="w2h")
                    nc.sync.dma_start(wh[:], w2T_h[li, kf * 128:(kf + 1) * 128, :])
                    w2_h.append(wh)
                    wl = w2_p.tile([128, D], F16, tag="w2", name="w2l")
                    nc.scalar.dma_start(wl[:], w2T_l[li, kf * 128:(kf + 1) * 128, :])
                    w2_l.append(wl)
                # rolling stats (shares the mm PSUM ring)
                bc_cur = ln_stats(0, ps_mm, "mm")
                for b_ in range(NB):
                    tsl = slice(b_ * TB, (b_ + 1) * TB)
                    h2 = ln_h(b_, bc_cur, "h")
                    if b_ + 1 < NB:
                        bc_cur = ln_stats(b_ + 1, ps_mm, "mm")
                    acc = [ps_acc.tile([128, TB], F32, tag="acc", name="facc")[:]
                           for _ in range(KD)]
                    rl_t = [None] * KF

                    def ff1(kf):
                        fsl = slice(kf * 128, (kf + 1) * 128)
                        ps = ps_mm.tile([128, TB], F32, tag="mm", name="ps")
                        for k in range(KD):
                            nc.tensor.matmul(ps[:], w1_h[k][:, fsl], h2[k][0][:],
                                             start=(k == 0), stop=False)
                        for k in range(KD):
                            nc.tensor.matmul(ps[:], w1_h[k][:, fsl], h2[k][1][:],
                                             start=False, stop=False)
                        for k in range(KD):
                            nc.tensor.matmul(ps[:], w1_l[k][:, fsl], h2[k][0][:],
                                             start=False, stop=(k == KD - 1))
                        rl_h = relu_p.tile([128, TB], F16, tag="rl_h", name="rl_h")
                        nc.scalar.activation(rl_h[:], ps[:], AF.Relu,
                                             bias=ccol(cb + 12 + kf), scale=1.0)
                        zz = hp.tile([128, TB], F32, tag="hf", name="zz")
                        nc.vector.tensor_scalar(zz[:], ps[:], ccol(cb + 12 + kf), 0.0,
                                                op0=ALU.add, op1=ALU.max)
                        rl_l = relu_p.tile([128, TB], F16, tag="rl_l", name="rl_l")
                        nc.vector.tensor_sub(rl_l[:], zz[:], rl_h[:])
                        return (rl_h, rl_l)

                    def ff2(kf):
                        rl_h, rl_l = rl_t[kf]
                        for ot in range(KD):
                            osl = slice(ot * 128, (ot + 1) * 128)
                            nc.tensor.matmul(acc[ot], w2_h[kf][:, osl], rl_h[:],
                                             start=(kf == 0), stop=False)
                            nc.tensor.matmul(acc[ot], w2_h[kf][:, osl], rl_l[:],
                                             start=False, stop=False)
                            nc.tensor.matmul(acc[ot], w2_l[kf][:, osl], rl_h[:],
                                             start=False, stop=(kf == KF - 1))

                    for kf in range(KF):
                        rl_t[kf] = ff1(kf)
                        if kf >= 1:
                            ff2(kf - 1)
                    ff2(KF - 1)
                    for ot in range(KD):
                        nc.vector.scalar_tensor_tensor(xT[ot][:, tsl], acc[ot],
                                                       ccol(cb + 28 + ot), xT[ot][:, tsl],
                                                       op0=ALU.add, op1=ALU.add)
        # ============ generator pools released here =========================

        # ---------------- matching-stage pools (reuse generator space) ------
        outw_p = ctx.enter_context(tc.tile_pool(name="outw", bufs=1))
        outp2 = ctx.enter_context(tc.tile_pool(name="outp2", bufs=1))
        mrow = ctx.enter_context(tc.tile_pool(name="mrow", bufs=1))
        mbcp = ctx.enter_context(tc.tile_pool(name="mbc", bufs=1))
        mtch = ctx.enter_context(tc.tile_pool(name="mtch", bufs=1))
        m2p = ctx.enter_context(tc.tile_pool(name="m2p", bufs=2))
        wT_p = ctx.enter_context(tc.tile_pool(name="wTp", bufs=8))
        pnat_p = ctx.enter_context(tc.tile_pool(name="pnat", bufs=1))
        xfa_p = ctx.enter_context(tc.tile_pool(name="xfa", bufs=1))
        xfTs_p = ctx.enter_context(tc.tile_pool(name="xfTs", bufs=2))
        pts_p = ctx.enter_context(tc.tile_pool(name="pts", bufs=4))
        xsp_p = ctx.enter_context(tc.tile_pool(name="xsp", bufs=2))

        # fp32 local xf in natural layout [scene, feature]
        xfl = outp2.tile([SC, FDIM], F32, tag="xfl", name="xfl")
        # y_T [ch, tok]: fp32 (for transposes) and bf16 (matching source)
        yT_f = outp2.tile([128, T], F32, tag="yTf")
        yT_bf = outp2.tile([128, T], BF16, tag="yTbf")

        # ========= output projection (f16 3-pass from xT splits) =========
        outw_h = outw_p.tile([128, KD * CH], F16, tag="outwh", name="outwh")
        outw_l = outw_p.tile([128, KD * CH], F16, tag="outwl", name="outwl")
        for k in range(KD):
            nc.sync.dma_start(outw_h[:, k * CH:(k + 1) * CH],
                              outwT_h[k * 128:(k + 1) * 128, :])
            nc.sync.dma_start(outw_l[:, k * CH:(k + 1) * CH],
                              outwT_l[k * 128:(k + 1) * 128, :])
        outb_col = ccol(4 + 32 * DEPTH)

        for b_ in range(NB):
            bsl = slice(b_ * TB, (b_ + 1) * TB)
            xs = []
            for k in range(KD):
                xh = xsp_p.tile([128, TB], F16, tag=f"xh{k}", name="xh")
                nc.scalar.activation(xh[:], xT[k][:, bsl], AF.Identity, bias=0.0, scale=1.0)
                xl = xsp_p.tile([128, TB], F16, tag=f"xl{k}", name="xl")
                nc.vector.tensor_sub(xl[:], xT[k][:, bsl], xh[:])
                xs.append((xh, xl))
            ps = ps_mm.tile([128, TB], F32, tag="mm", name="ps")
            for k in range(KD):
                nc.tensor.matmul(ps[:], outw_h[:, k * CH:(k + 1) * CH], xs[k][0][:],
                                 start=(k == 0), stop=False)
            for k in range(KD):
                nc.tensor.matmul(ps[:], outw_h[:, k * CH:(k + 1) * CH], xs[k][1][:],
                                 start=False, stop=False)
            for k in range(KD):
                nc.tensor.matmul(ps[:], outw_l[:, k * CH:(k + 1) * CH], xs[k][0][:],
                                 start=False, stop=(k == KD - 1))
            nc.scalar.activation(yT_f[:, bsl], ps[:], AF.Identity,
                                 bias=outb_col, scale=1.0)
            nc.vector.tensor_copy(yT_bf[:, bsl], yT_f[:, bsl])

        # y natural via PE transposes of yT_f; bf16 shards into ag_in
        for tt in range(T // 128):
            tp = ps_at.tile([128, CH], F32, tag="at", name="yn_tp")
            nc.tensor.transpose(tp[:], yT_f[:, tt * 128:(tt + 1) * 128], ident[:])
            yn = m2p.tile([128, CH], F32, tag="yn", name="yn")
            nc.vector.tensor_copy(yn[:], tp[:])
            nc.sync.dma_start(
                xfl[tt * 4:(tt + 1) * 4, :].rearrange("p (l c) -> p l c", l=L),
                yn[:])
            ynbf = m2p.tile([128, CH], BF16, tag="ynbf", name="ynbf")
            nc.vector.tensor_copy(ynbf[:], yn[:])
            nc.scalar.dma_start(
                ag_in[AG_XFN + tt * 4 * FDIM: AG_XFN + (tt + 1) * 4 * FDIM]
                .rearrange("(i l c) -> i l c", l=L, c=CH),
                ynbf[:])

        # xn = ||xf_i||^2 via gram diag (bf16 inputs, fp32 accum)
        xfT_st = yT_bf[:].rearrange("c (i l) -> c l i", l=L)   # [128, 32, 64]
        gram = ps_at.tile([SC, SC], F32, tag="at", name="gram")
        for l in range(KFl):
            nc.tensor.matmul(gram[:], xfT_st[:, l, :], xfT_st[:, l, :],
                             start=(l == 0), stop=(l == KFl - 1))
        gd = m2p.tile([SC, SC], F32, tag="gd", name="gd")
        nc.vector.tensor_mul(gd[:], gram[:], ident[0:SC, 0:SC])
        xn_col = colp.tile([SC, 1], F32, tag="xncol", name="xncol")
        nc.vector.reduce_sum(xn_col[:], gd[:], axis=AX.X)
        agi_f32 = ag_in_u.bitcast(F32)
        nc.sync.dma_start(
            agi_f32[AG_XN // 2:AG_XN // 2 + SC].rearrange("(i bb) -> i bb", bb=1),
            xn_col[:])
        nc.gpsimd.collective_compute(
            "AllGather", ALU.bypass, replica_groups=[list(range(NC_))],
            ins=[ag_in_u[:]], outs=[ag_out_u[:]])

        # preload p tiles (no dependence on AG)
        pnat_t = []
        for jt in range(4):
            t = pnat_p.tile([128, FDIM], BF16, tag=f"pn{jt}", name=f"pn{jt}")
            nc.scalar.dma_start(t[:], pnat[jt * 128:(jt + 1) * 128, :])
            pnat_t.append(t)

        # S_pos (does not need AG): acc over 32 f-chunks
        spos = ps_acc.tile([SC, B], F32, tag="acc", name="spos")
        for l in range(KFl):
            mv = pts_p.tile([128, B], BF16, tag="mv", name="mv")
            nc.sync.dma_start(mv[:], pT[l * 128:(l + 1) * 128, :])
            nc.tensor.matmul(spos[:], xfT_st[:, l, :], mv[:],
                             start=(l == 0), stop=(l == KFl - 1))

        # xn_full row [1, 512] f32 + broadcast
        ago_f32 = ag_out_u.bitcast(F32)
        xn_row = mrow.tile([1, B], F32, tag="mr", name="xnrow")
        nc.sync.dma_start(
            xn_row[:],
            bass.AP(tensor=ago_f32.tensor, offset=ago_f32.offset + AG_XN // 2,
                    ap=[[1, 1], [AG_SZ // 2, NC_], [1, SC]]))
        xn_bc = mbcp.tile([SC, B], F32, tag="mbc", name="xnbc")
        nc.gpsimd.partition_broadcast(xn_bc[:], xn_row[:])

        # xf_nat_all: 4 scene-tiles [128, 4096] bf16 (8KB lines)
        xfa = []
        for st in range(4):
            t = xfa_p.tile([128, FDIM], BF16, tag=f"xfa{st}", name=f"xfa{st}")
            for half in range(2):
                c = 2 * st + half
                nc.sync.dma_start(
                    t[half * SC:(half + 1) * SC, :],
                    bass.AP(tensor=ag_out.tensor,
                            offset=ag_out.offset + c * AG_SZ + AG_XFN,
                            ap=[[FDIM, SC], [1, FDIM]]))
            xfa.append(t)

        # S_neg: rebuild xf^T_all [128 f, 512 scene] per l-chunk via PE
        # transposes (double-buffered), accumulate immediately
        sneg = ps_acc.tile([SC, B], F32, tag="acc", name="sneg")
        for l in range(KFl):
            xfT_l = xfTs_p.tile([128, B], BF16, tag="xfTs", name="xfTs")
            for st in range(4):
                tp = ps_at.tile([128, 128], BF16, tag="at", name="ttp")
                nc.tensor.transpose(tp[:], xfa[st][:, l * 128:(l + 1) * 128],
                                    ident_bf[:])
                nc.vector.tensor_copy(xfT_l[:, st * 128:(st + 1) * 128], tp[:])
            nc.tensor.matmul(sneg[:], xfT_st[:, l, :], xfT_l[:],
                             start=(l == 0), stop=(l == KFl - 1))

        # distances -> logits -> E (in place); pos half first (no AG dep)
        dist = mtch.tile([SC, 2 * B], F32, tag="dist")
        nc.vector.scalar_tensor_tensor(dist[:, 0:B], spos[:], -2.0, pn_t[:],
                                       op0=ALU.mult, op1=ALU.add)
        nc.vector.tensor_scalar_add(dist[:, 0:B], dist[:, 0:B], xn_col[:])
        nc.vector.tensor_scalar_max(dist[:, 0:B], dist[:, 0:B], 0.0)
        nc.scalar.activation(dist[:, 0:B], dist[:, 0:B], AF.Sqrt, bias=0.0, scale=1.0)
        nc.vector.scalar_tensor_tensor(dist[:, B:2 * B], sneg[:], -2.0, xn_bc[:],
                                       op0=ALU.mult, op1=ALU.add)
        nc.vector.tensor_scalar_add(dist[:, B:2 * B], dist[:, B:2 * B], xn_col[:])
        nc.vector.tensor_scalar_max(dist[:, B:2 * B], dist[:, B:2 * B], 0.0)
        nc.scalar.activation(dist[:, B:2 * B], dist[:, B:2 * B], AF.Sqrt, bias=0.0, scale=1.0)
        nc.vector.tensor_add(dist[:, B:2 * B], dist[:, B:2 * B], nd_t[:])
        dmin = colp.tile([SC, 1], F32, tag="dmin", name="dmin")
        nc.vector.tensor_reduce(out=dmin[:], in_=dist[:], axis=AX.X, op=ALU.min)
        E = dist  # in place: E = exp(-d + dmin)
        nc.scalar.activation(E[:], dist[:], AF.Exp, bias=dmin[:], scale=-1.0)
        g_col = colp.tile([SC, 1], F32, tag="gcol", name="gcol")
        nc.scalar.activation(g_col[:], dmin[:], AF.Exp, bias=m20_col[:], scale=-1.0)
        sr_col = colp.tile([SC, 1], F32, tag="srcol", name="srcol")
        nc.vector.reduce_sum(sr_col[:], E[:], axis=AX.X)
        # partial colsums of G = E * g_i via g-weighted stationary
        cs_row = mrow.tile([1, 2 * B], F32, tag="mr", name="csrow")
        for b_ in range(2):
            ps = ps_mm.tile([1, B], F32, tag="mm", name="ps")
            nc.tensor.matmul(ps[:], g_col[:], E[:, b_ * B:(b_ + 1) * B],
                             start=True, stop=True)
            nc.vector.tensor_copy(cs_row[:, b_ * B:(b_ + 1) * B], ps[:])
        nc.sync.dma_start(ar_in, cs_row[:])
        nc.gpsimd.collective_compute(
            "AllReduce", ALU.add, replica_groups=[list(range(NC_))],
            ins=[ar_in[:]], outs=[ar_out[:]])
        cs_g = mrow.tile([1, 2 * B], F32, tag="mr", name="csg")
        nc.sync.dma_start(cs_g[:], ar_out)
        cs_bc = mbcp.tile([SC, 2 * B], F32, tag="csbc", name="csbc")
        nc.gpsimd.partition_broadcast(cs_bc[:], cs_g[:])
        nc.scalar.activation(cs_bc[:], cs_bc[:], AF.Sqrt, bias=0.0, scale=1.0)
        nc.vector.reciprocal(cs_bc[:], cs_bc[:])
        # E' = E * invsqrt(Sc); row scalars BEFORE overwriting E with W
        nc.vector.tensor_mul(E[:], E[:], cs_bc[:])
        snp = colp.tile([SC, 1], F32, tag="snp", name="snp")
        nc.vector.reduce_sum(snp[:], E[:, B:2 * B], axis=AX.X)
        spp = colp.tile([SC, 1], F32, tag="spp", name="spp")
        nc.vector.reduce_sum(spp[:], E[:, 0:B], axis=AX.X)
        tcol = colp.tile([SC, 1], F32, tag="tcol", name="tcol")
        nc.vector.reciprocal(tcol[:], sr_col[:])
        nc.vector.tensor_mul(tcol[:], tcol[:], g_col[:])
        ccl = colp.tile([SC, 1], F32, tag="ccol", name="ccol")
        nc.scalar.activation(ccl[:], tcol[:], AF.Sqrt, bias=0.0, scale=1.0)
        alpha = colp.tile([SC, 1], F32, tag="alpha", name="alpha")
        nc.vector.tensor_mul(alpha[:], tcol[:], snp[:])
        beta = colp.tile([SC, 1], F32, tag="beta", name="beta")
        nc.vector.tensor_mul(beta[:], alpha[:], spp[:])
        nc.vector.tensor_mul(beta[:], beta[:], ccl[:])
        nc.vector.tensor_scalar_mul(beta[:], beta[:], -1.0)
        # W = E' * alpha / -beta (in place), transpose, cast bf16
        nc.vector.tensor_scalar_mul(E[:, 0:B], E[:, 0:B], alpha[:])
        nc.vector.tensor_scalar_mul(E[:, B:2 * B], E[:, B:2 * B], beta[:])
        wT = []
        for half in range(2):
            for jt in range(4):
                tp = ps_at.tile([128, SC], F32, tag="at", name="wtp")
                nc.tensor.transpose(
                    tp[:], E[:, half * B + jt * 128: half * B + (jt + 1) * 128],
                    ident[0:SC, 0:SC])
                t = wT_p.tile([128, SC], BF16, tag="wT", name="wT")
                nc.vector.tensor_copy(t[:], tp[:])
                wT.append(t)
        # V and loss: V = Wpos @ p - Wneg @ xf_full, r = xf - fl(xf + V)
        # everything SBUF-resident
        lacc = m2p.tile([SC, 8], F32, tag="lacc", name="lacc", bufs=1)
        FBW = 512
        for fb in range(FDIM // FBW):
            fsl = slice(fb * FBW, (fb + 1) * FBW)
            vps = ps_acc.tile([SC, FBW], F32, tag="acc", name="vps")
            for jt in range(4):
                nc.tensor.matmul(vps[:], wT[jt][:], pnat_t[jt][:, fsl],
                                 start=(jt == 0), stop=False)
            for jt in range(4):
                nc.tensor.matmul(vps[:], wT[4 + jt][:], xfa[jt][:, fsl],
                                 start=False, stop=(jt == 3))
            t1 = m2p.tile([SC, FBW], F32, tag="t1", name="t1")
            nc.vector.tensor_add(t1[:], xfl[:, fsl], vps[:])
            nc.vector.tensor_sub(t1[:], xfl[:, fsl], t1[:])
            nc.vector.tensor_mul(t1[:], t1[:], t1[:])
            nc.vector.reduce_sum(lacc[:, fb:fb + 1], t1[:], axis=AX.X)
        lsum = colp.tile([SC, 1], F32, tag="lsum", name="lsum")
        nc.vector.reduce_sum(lsum[:], lacc[:], axis=AX.X)
        tot = ps_mm.tile([1, 1], F32, tag="mm", name="tot")
        nc.tensor.matmul(tot[:], ones_col[0:SC, :], lsum[:], start=True, stop=True)
        tot_sb = colp.tile([1, 1], F32, tag="tot", name="totsb")
        nc.vector.tensor_copy(tot_sb[:], tot[:])
        nc.sync.dma_start(loss_part, tot_sb[:])

    nc.compile()
    return nc


_NC_CACHE = None


def _get_nc():
    global _NC_CACHE
    if _NC_CACHE is None:
        _NC_CACHE = _build_nc()
    return _NC_CACHE


def _split16(a):
    """f16 hi/lo split (round-to-nearest): a ~= hi + lo."""
    hi = a.astype(np.float16)
    lo = (a - hi.astype(np.float32)).astype(np.float16)
    return np.ascontiguousarray(hi), np.ascontiguousarray(lo)


def _prep_inputs(inputs):
    f32 = lambda x: np.ascontiguousarray(np.asarray(x), dtype=np.float32)
    bf = lambda x: np.ascontiguousarray(np.asarray(x, dtype=ml_dtypes.bfloat16))
    sample_p = f32(inputs["sample_p"])
    eps = f32(inputs["eps"])
    p2 = sample_p.reshape(B, FDIM)
    pn = (p2.astype(np.float64) ** 2).sum(-1).astype(np.float32)

    g1 = f32(inputs["ln1_g"])   # [DEPTH, D]
    b1n = f32(inputs["ln1_b"])
    g2 = f32(inputs["ln2_g"])
    b2n = f32(inputs["ln2_b"])
    Wqkv = f32(inputs["Wqkv"])  # [DEPTH, 3D, D]
    W1 = f32(inputs["W1"])      # [DEPTH, FF, D]

    # fold LN gamma into weights, LN beta into biases
    Gqkv = Wqkv * g1[:, None, :]
    bqkv_eff = f32(inputs["bqkv"]) + np.einsum('dij,dj->di', Wqkv, b1n)
    G1 = W1 * g2[:, None, :]
    b1_eff = f32(inputs["b1"]) + np.einsum('dij,dj->di', W1, b2n)

    # packed bias columns [128, NCOLS]
    colsP = np.zeros((128, NCOLS), np.float32)
    inb = f32(inputs["in_b"])
    for k in range(KD):
        colsP[:, k] = inb[k * 128:(k + 1) * 128]
    for li in range(DEPTH):
        cb = 4 + 32 * li
        for ot in range(8):
            colsP[:, cb + ot] = bqkv_eff[li, ot * 128:(ot + 1) * 128]
        for k in range(KD):
            colsP[:, cb + 8 + k] = f32(inputs["bo"])[li, k * 128:(k + 1) * 128]
            colsP[:, cb + 28 + k] = f32(inputs["b2"])[li, k * 128:(k + 1) * 128]
        for kf in range(KF):
            colsP[:, cb + 12 + kf] = b1_eff[li, kf * 128:(kf + 1) * 128]
    colsP[:, 4 + 32 * DEPTH] = f32(inputs["out_b"])

    inwT_h, inwT_l = _split16(f32(inputs["in_w"]).T.copy())
    wqkvT_h, wqkvT_l = _split16(np.ascontiguousarray(Gqkv.transpose(0, 2, 1)))
    woT_h, woT_l = _split16(np.ascontiguousarray(f32(inputs["Wo"]).transpose(0, 2, 1)))
    w1T_h, w1T_l = _split16(np.ascontiguousarray(G1.transpose(0, 2, 1)))
    w2T_h, w2T_l = _split16(np.ascontiguousarray(f32(inputs["W2"]).transpose(0, 2, 1)))
    outwT_h, outwT_l = _split16(f32(inputs["out_w"]).T.copy())

    common = {
        "inwT_h": inwT_h, "inwT_l": inwT_l,
        "wqkvT_h": wqkvT_h, "wqkvT_l": wqkvT_l,
        "bqkv": bqkv_eff,
        "woT_h": woT_h, "woT_l": woT_l,
        "w1T_h": w1T_h, "w1T_l": w1T_l,
        "w2T_h": w2T_h, "w2T_l": w2T_l,
        "outwT_h": outwT_h, "outwT_l": outwT_l,
        "colsP": colsP,
        "pT": bf(p2.T),
        "pnat": bf(p2),
        "pn_bc": np.broadcast_to(pn[None, :], (SC, B)).copy(),
        "attn_mask": np.tile(np.kron(np.eye(4, dtype=np.float32), np.ones((32, 32), np.float32)), (1, 4)),
    }
    in_maps = []
    for c in range(NC_):
        nd = np.zeros((SC, B), np.float32)
        nd[np.arange(SC), SC * c + np.arange(SC)] = 1e6
        m = dict(common)
        eT = eps[c * SC:(c + 1) * SC].reshape(T, CH).T.copy()
        eh, el = _split16(eT)
        m["epsT_h"] = eh
        m["epsT_l"] = el
        m["negdiag"] = nd
        in_maps.append(m)
    return in_maps


def kernel(**inputs) -> np.ndarray:
    nc = _get_nc()
    in_maps = _prep_inputs(inputs)
    res = run_bass_kernel_spmd(nc, in_maps, list(range(NC_)))
    total = sum(float(r["loss_part"][0, 0]) for r in res.results)
    return np.float32(total / (B * FDIM))
